# revision 20
# baseline (speedup 1.0000x reference)
"""CrossBlock (cross-attention transformer block) on 8 TRN2 NeuronCores.

Sharding: 4 batch elements x 2 cores each (tensor-parallel over heads).
Core c = 2*b + half handles batch b; half selects heads 6*half..6*half+5.

v2: fp8(e4m3) DoubleRow matmuls everywhere (2 k-tiles per instruction),
softmax exp split between the Scalar (ACT) engine (true exp) and the Vector
(DVE) engine (one-instruction cubic-poly exp via a custom DVE op), psum->sbuf
copies spread over ACT/DVE, sbuf-only elementwise work on GPSIMD (Pool).
LayerNorm rstd via exp(-0.5*ln(var+eps)) so the attention phase stays within
one ACT function table. Weights are host-scaled by SW=32 to keep fp8 operands
out of the subnormal range; the inverse scales fold into activation-scale /
affine ops. Biases that only shift logits uniformly per query (k bias) are
dropped; v bias is folded into the proj bias on the host.

Per-core flow:
  LN1 stats (fp8 ones-matmuls) -> Q/K/V projections (fp8 DR) -> attention
  (S^T fp8 DR with a zero k-slot; exp ACT/DVE split; P@V fp8 DR with a ones
  column for the softmax denominator; normalize via tensor-tensor divide with
  a stride-0 denominator view) -> proj partial -> pairwise fp8 ReduceScatter
  (token split) -> residual + LN2 + MLP (fp8 DR, gelu on ACT, bias via a
  K=1 matmul) -> f32 output.
"""

import numpy as np
import ml_dtypes
from contextlib import ExitStack

import concourse.bass as bass
import concourse.tile as tile
from concourse import bacc, mybir
from concourse.bass_utils import run_bass_kernel_spmd
from concourse.masks import make_identity

F32 = mybir.dt.float32
BF16 = mybir.dt.bfloat16
FP8 = mybir.dt.float8e4
AF = mybir.ActivationFunctionType
ALU = mybir.AluOpType
DRM = mybir.MatmulPerfMode.DoubleRow
BF = ml_dtypes.bfloat16
F8 = ml_dtypes.float8_e4m3

DIM = 768
NH = 12
HD = 64
MLPD = 3072
EPS = 1e-5
B = 4
TQ = 1024          # query tokens per batch element
TKV = 4096         # kv tokens per batch element
NHL = NH // 2      # heads per core (6)
DL = NHL * HD      # local head cols (384)
TH = TQ // 2       # token half for the MLP stage (512)
CT = DIM // 128    # channel tiles (6)
CP = CT // 2       # channel pairs (3)
DT = DL // 128     # local head-pair groups (3)
HT = MLPD // 128   # hidden tiles (24)
HP = HT // 2       # hidden pairs (12)
KTT = TKV // 128   # kv token tiles (32)
NCORES = 8

SW = 32.0                      # host-side fp8 weight scale
ASC = HD ** -0.5               # attention scale (1/8)
ALPHA = ASC / (SW * SW)        # fold of attn scale + q/k weight scales
INV_SW = 1.0 / SW
INV_SW2 = 1.0 / (SW * SW)

# exp split: one ACT (true exp) per EXP_RATIO kt-groups, rest DVE poly.
EXP_ACT_OF = 8     # of every 16 groups, this many go to ACT

_CACHE = {}

# ---------------------------------------------------------------------------
# custom DVE op: one-instruction cubic exp approximation
#   f(s) = C0*s^3 + C1*s^2 + imm2*s + 1  (= Taylor of exp(imm2*s) when
#   C0=imm2^3/6, C1=imm2^2/2). The constant term 1 is exact, which keeps
#   softmax normalization consistent with the ACT-exp share.
# ---------------------------------------------------------------------------
import concourse.dve_ops as dve_ops
from concourse.dve_spec import Spec, Src0, C0, C1, C2, One, lower as dve_lower
from concourse.dve_uop import DveOpSpec


def _register_exp_poly():
    if hasattr(dve_ops, "_EXP_POLY3_OP"):
        return dve_ops._EXP_POLY3_OP
    body = ((Src0 * C0 + C1) * Src0 + C2) * Src0 + One
    spec = Spec(
        body=body,
        reference=lambda in0, in1, c0, c1, c2: (
            ((in0.astype(np.float32) * c0 + c1) * in0 + c2) * in0 + 1.0
        ),
    )
    name = "EXP_POLY3"
    opcode = dve_ops._CUSTOM_DVE_ROW_BASE + len(dve_ops.OPS)
    shas = {}
    for ver in ("v3", "v4"):
        s = DveOpSpec(name=name, opcode=opcode, uops=dve_lower(spec, ver=ver),
                      rd1_en=False)
        shas[ver] = s.sha(ver)
    op = dve_ops.DveOp(name, spec, subdim=False, uops_sha=shas)
    dve_ops.OPS.append(op)
    dve_ops._SUB_OPCODE_FOR_NAME[name] = opcode
    dve_ops.CUSTOM_DVE_SPECS[name] = spec
    dve_ops._EXP_POLY3_OP = op
    return op


EXP_POLY3 = _register_exp_poly()
P3_C0 = ALPHA ** 3 / 6.0
P3_C1 = ALPHA ** 2 / 2.0


def _build_program():
    nc = bacc.Bacc("TRN2", target_bir_lowering=False, debug=False,
                   num_devices=NCORES)

    din = {}

    def inp(name, shape, dt):
        din[name] = nc.dram_tensor(name, list(shape), dt,
                                   kind="ExternalInput").ap()
        return din[name]

    xq8_d = inp("xq8", (128, CP, 2, TQ), FP8)
    xqh_d = inp("xqh", (128, CT, TH), BF16)
    xkv8_d = inp("xkv8", (128, CP, 2, TKV), FP8)
    wq8_d = inp("wq8", (128, CP, 2, DL), FP8)
    wk8_d = inp("wk8", (128, CP, 2, DL), FP8)
    wv8_d = inp("wv8", (128, CP, 2, DL), FP8)
    wp8_d = inp("wp8", (128, 2, 2, DIM), FP8)
    wm18_d = inp("wm18", (128, CP, 2, MLPD), FP8)
    wm28_d = inp("wm28", (128, HP, 2, DIM), FP8)
    b1p8_d = inp("b1p8", (1, HT, 2, 128), FP8)
    sq_d = inp("sq", (DL,), F32)
    qb_d = inp("qb", (DL,), F32)
    pb_d = inp("pb", (DIM,), F32)
    b2_d = inp("b2", (DIM,), F32)
    out_d = nc.dram_tensor("out", [DIM, TH], F32, kind="ExternalOutput").ap()

    cc_in = [nc.dram_tensor(f"cc_in{i}", [2, DIM, TH // 2], FP8).ap()
             for i in range(2)]
    cc_rs = [nc.dram_tensor(f"cc_rs{i}", [DIM, TH // 2], FP8).ap()
             for i in range(2)]
    groups = [[0, 1], [2, 3], [4, 5], [6, 7]]

    with tile.TileContext(nc) as tc, ExitStack() as ctx:
        # ---- pools ----
        const = ctx.enter_context(tc.tile_pool(name="const", bufs=1))
        big = ctx.enter_context(tc.tile_pool(name="big", bufs=1))
        kvp = ctx.enter_context(tc.tile_pool(name="kvp", bufs=1))
        sexp_pool = ctx.enter_context(tc.tile_pool(name="sexp", bufs=9))
        med = ctx.enter_context(tc.tile_pool(name="med", bufs=1))
        sm = ctx.enter_context(tc.tile_pool(name="sm", bufs=2))
        sm2 = ctx.enter_context(tc.tile_pool(name="sm2", bufs=3))
        upool = ctx.enter_context(tc.tile_pool(name="upool", bufs=2))
        outp = ctx.enter_context(tc.tile_pool(name="outp", bufs=2))
        bc_pool = ctx.enter_context(tc.tile_pool(name="bc", bufs=2))

        # PSUM: ppS 2x2 banks (S pairs / MLP h), ppo 2x1 (PV accum),
        # ppK 2x1 (K/V/Q/proj/MLP2/stats)
        ppS = ctx.enter_context(tc.tile_pool(name="ppS", bufs=3, space="PSUM"))
        ppo = ctx.enter_context(tc.tile_pool(name="ppo", bufs=1, space="PSUM"))
        ppK = ctx.enter_context(tc.tile_pool(name="ppK", bufs=1, space="PSUM"))

        # ---- constants ----
        ones_bf = const.tile([128, 1], BF16)
        nc.vector.memset(ones_bf, 1.0)
        ones8_t = const.tile([128, 2, 16], FP8)
        nc.vector.memset(ones8_t, 1.0)
        ones8 = ones8_t[:, :, 0:1]
        ones8r = const.tile([1, 2, 256], FP8)
        nc.vector.memset(ones8r, 1.0)
        ident = const.tile([128, 128], BF16)
        make_identity(nc, ident)
        eps_t = const.tile([1, 1], F32)
        nc.vector.memset(eps_t, EPS)

        # ---- resident inputs / weights ----
        xq8 = big.tile([128, CP, 2, TQ], FP8, tag="xq8")
        nc.sync.dma_start(xq8, xq8_d)
        xkv8 = big.tile([128, CP, 2, TKV], FP8, tag="xkv8")
        nc.sync.dma_start(xkv8[:, :, :, 0:2048],
                          xkv8_d[:, :, :, 0:2048])
        nc.sync.dma_start(xkv8[:, :, :, 2048:TKV],
                          xkv8_d[:, :, :, 2048:TKV])
        wq8 = const.tile([128, CP, 2, DL], FP8)
        nc.sync.dma_start(wq8, wq8_d)
        wk8 = const.tile([128, CP, 2, DL], FP8)
        nc.sync.dma_start(wk8, wk8_d)
        wv8 = const.tile([128, CP, 2, DL], FP8)
        nc.sync.dma_start(wv8, wv8_d)
        xqh_sb = big.tile([128, CT, TH], BF16, tag="xqh")
        nc.sync.dma_start(xqh_sb, xqh_d)
        wp8 = const.tile([128, 2, 2, DIM], FP8)
        nc.sync.dma_start(wp8, wp8_d)
        wm18 = big.tile([128, CP, 2, MLPD], FP8, tag="wm1")
        nc.sync.dma_start(wm18, wm18_d)
        wm18r_d = nc.dram_tensor(
            "wm18r", [128, CP, 2, MLPD], FP8, kind="ExternalInput").ap()
        wm28r_d = nc.dram_tensor(
            "wm28r", [128, HP, 2, DIM], FP8, kind="ExternalInput").ap()
        b1p8 = const.tile([1, HT, 2, 128], FP8)
        nc.sync.dma_start(b1p8, b1p8_d)
        sq_sb = const.tile([128, DT], F32)
        nc.sync.dma_start(sq_sb, sq_d.rearrange("(a p) -> p a", p=128))
        qb_sb = const.tile([128, DT], F32)
        nc.sync.dma_start(qb_sb, qb_d.rearrange("(a p) -> p a", p=128))
        pb_sb = const.tile([128, CT], F32)
        nc.sync.dma_start(pb_sb, pb_d.rearrange("(a p) -> p a", p=128))
        b2_sb = const.tile([128, CT], F32)
        nc.sync.dma_start(b2_sb, b2_d.rearrange("(a p) -> p a", p=128))

        # ---- persistent attention tiles ----
        # K^T per d-group: [128(dl of 2 heads), TKV] fp8
        kt_sbs = [kvp.tile([128, TKV], FP8, tag=f"kt{d}", name=f"kt{d}")
                  for d in range(DT)]
        # V per d-group: [128(kt), group(16), slot(2), head(2), HD+1] fp8
        v_sbs = [kvp.tile([128, KTT // 2, 2, 2, HD + 1], FP8, tag=f"v{d}",
                          name=f"v{d}")
                 for d in range(DT)]
        # Q^T per d-group: [128(dl), slot(2), TQ] fp8, slot1 = zeros
        qt_sbs = [kvp.tile([128, 2, TQ], FP8, tag=f"qt{d}", name=f"qt{d}")
                  for d in range(DT)]
        for d in range(DT):
            nc.gpsimd.memset(qt_sbs[d][:, 1, :], 0.0)
            nc.gpsimd.memset(v_sbs[d][:, :, :, :, HD:HD + 1], 1.0)

        o_sb = med.tile([128, 8, DL], BF16, tag="osb")       # normalized O
        ot_sb = med.tile([128, DT, TQ], FP8, tag="ot")       # O^T for proj
        x1_sb = med.tile([128, CT, TQ], FP8, tag="x1")       # proj partial
        x1h = med.tile([128, CT, TH], BF16, tag="x1h")       # post-RS resid

        # ================= LN1 stats (from fp8 xq) ========================
        xsq8 = big.tile([128, CP, 2, TQ], FP8, tag="wm1r", name="xsq8")
        for p in range(CP):
            nc.gpsimd.tensor_tensor(xsq8[:, p], xq8[:, p], xq8[:, p],
                                    op=ALU.mult)

        mu_row = sm.tile([1, TQ], BF16, tag="st1")
        rs_row = sm.tile([1, TQ], BF16, tag="st1")
        for t2 in range(2):
            tsl = slice(t2 * 512, (t2 + 1) * 512)
            s_ps = ppK.tile([1, 512], F32, tag="k", name=f"sps{t2}")
            for p in range(CP):
                nc.tensor.matmul(s_ps, ones8, xq8[:, p, :, tsl],
                                 start=(p == 0), stop=(p == CP - 1),
                                 perf_mode=DRM)
            nc.vector.tensor_scalar_mul(mu_row[:, tsl], s_ps, 1.0 / DIM)
            q_ps = ppK.tile([1, 512], F32, tag="k", name=f"qps{t2}")
            for p in range(CP):
                nc.tensor.matmul(q_ps, ones8, xsq8[:, p, :, tsl],
                                 start=(p == 0), stop=(p == CP - 1),
                                 perf_mode=DRM)
            m2 = sm2.tile([1, 512], F32, tag="st2", name=f"m2{t2}")
            nc.vector.tensor_tensor(m2, mu_row[:, tsl], mu_row[:, tsl],
                                    op=ALU.mult)
            var = sm2.tile([1, 512], F32, tag="st2", name=f"var{t2}")
            nc.vector.scalar_tensor_tensor(var, q_ps, 1.0 / DIM, m2,
                                           op0=ALU.mult, op1=ALU.subtract)
            lnv = sm2.tile([1, 512], F32, tag="st2", name=f"lnv{t2}")
            nc.scalar.activation(lnv, var, AF.Ln, bias=eps_t[:1, :], scale=1.0)
            nc.scalar.activation(rs_row[:, tsl], lnv, AF.Exp, scale=-0.5)
        mu_b = bc_pool.tile([128, TQ], BF16, tag="bc")
        nc.gpsimd.partition_broadcast(mu_b, mu_row)
        rs_b = bc_pool.tile([128, TQ], BF16, tag="bc")
        nc.gpsimd.partition_broadcast(rs_b, rs_row)
        # wm18r reuses xsq8's slot (tag wm1r); DMA lands after stats read it
        wm18r = big.tile([128, CP, 2, MLPD], FP8, tag="wm1r", name="wm18r")
        nc.sync.dma_start(wm18r, wm18r_d)

        # ================= projections ====================================
        def make_q(d):
            dsl = slice(d * 128, (d + 1) * 128)
            qt = qt_sbs[d]
            for t2 in range(2):
                tsl = slice(t2 * 512, (t2 + 1) * 512)
                y_ps = ppK.tile([128, 512], F32, tag="k", name=f"y{d}{t2}")
                for p in range(CP):
                    nc.tensor.matmul(y_ps, wq8[:, p, :, dsl],
                                     xq8[:, p, :, tsl], start=(p == 0),
                                     stop=(p == CP - 1), perf_mode=DRM)
                u = upool.tile([128, 512], F32, tag="u", name=f"u{d}{t2}")
                nc.vector.scalar_tensor_tensor(u, mu_b[:, tsl],
                                               sq_sb[:, d:d + 1], y_ps,
                                               op0=ALU.mult, op1=ALU.subtract)
                v2 = upool.tile([128, 512], F32, tag="u", name=f"v{d}{t2}")
                nc.gpsimd.tensor_tensor(v2, u, rs_b[:, tsl], op=ALU.mult)
                nc.vector.tensor_scalar(qt[:, 0, tsl], v2, scalar1=-1.0,
                                        op0=ALU.mult,
                                        scalar2=qb_sb[:, d:d + 1],
                                        op1=ALU.add)

        def make_kv(d):
            dsl = slice(d * 128, (d + 1) * 128)
            kt, v_sb = kt_sbs[d], v_sbs[d]
            for ch in range(TKV // 512):
                ksl = slice(ch * 512, (ch + 1) * 512)
                k_ps = ppK.tile([128, 512], F32, tag="k", name=f"k{d}{ch}")
                for p in range(CP):
                    nc.tensor.matmul(k_ps, wk8[:, p, :, dsl],
                                     xkv8[:, p, :, ksl], start=(p == 0),
                                     stop=(p == CP - 1), perf_mode=DRM)
                nc.scalar.copy(kt[:, ksl], k_ps)
                v_ps = ppK.tile([128, 4, 128], F32, tag="k", name=f"vp{d}{ch}")
                for j in range(4):
                    ktt = ch * 4 + j
                    ktsl = slice(ktt * 128, (ktt + 1) * 128)
                    for p in range(CP):
                        nc.tensor.matmul(v_ps[:, j, :],
                                         xkv8[:, p, :, ktsl],
                                         wv8[:, p, :, dsl], start=(p == 0),
                                         stop=(p == CP - 1), perf_mode=DRM)
                # [128,4,128] -> v_sb[:, 2ch:2ch+2, :, :, 0:HD]
                nc.scalar.copy(
                    v_sb[:, 2 * ch:2 * ch + 2, :, :, 0:HD],
                    v_ps.rearrange("q (g s) (h x) -> q g s h x", g=2, h=2))

        # ================= attention ======================================
        def attn_one(d, hh, t2):
            qt, kt, v_sb = qt_sbs[d], kt_sbs[d], v_sbs[d]
            tsl = slice(t2 * 512, (t2 + 1) * 512)
            rsl = slice(hh * 64, hh * 64 + 64)
            o_ps = ppo.tile([128, 4, HD + 1], F32, tag="o",
                            name=f"ops{d}{hh}{t2}")
            q_dr = qt[rsl, :, tsl]
            NW = 8                      # groups per wave
            for w in range(KTT // 2 // NW):
                sexps = []
                for gg in range(NW):
                    g = w * NW + gg
                    s_ps = ppS.tile([128, 2, 512], F32, tag="s",
                                    name=f"s{d}{hh}{t2}{g}")
                    for i in range(2):
                        ktt = 2 * g + i
                        kbase = kt[rsl, ktt * 128:(ktt + 1) * 128]
                        k_dr = bass.AP(tensor=kbase.tensor,
                                       offset=kbase.offset,
                                       ap=[kbase.ap[0], [0, 2], kbase.ap[1]])
                        nc.tensor.matmul(s_ps[:, i, :], k_dr, q_dr,
                                         start=True, stop=True, perf_mode=DRM)
                    sexp = sexp_pool.tile([128, 2, 512], FP8, tag="se",
                                          name=f"se{d}{hh}{t2}{g}")
                    sexps.append(sexp)
                    if g % 2 == 0:
                        nc.scalar.activation(sexp, s_ps, AF.Exp, scale=ALPHA)
                    else:
                        nc.vector._custom_dve(EXP_POLY3, out=sexp, in0=s_ps,
                                              s0=P3_C0, s1=P3_C1, imm2=ALPHA)
                for gg in range(NW):
                    g = w * NW + gg
                    for tt in range(4):
                        nc.tensor.matmul(
                            o_ps[:, tt, :],
                            sexps[gg][:, :, tt * 128:(tt + 1) * 128],
                            v_sb[:, g, :, hh, :],
                            start=(g == 0), stop=(g == KTT // 2 - 1),
                            perf_mode=DRM)
            # normalize: o = o_raw * (1/denom), stride-0 reciprocal view
            rr = sm.tile([128, 4], F32, tag="rr", name=f"rr{d}{hh}{t2}")
            nc.vector.reciprocal(rr, o_ps[:, :, HD])
            rrv = bass.AP(tensor=rr.tensor, offset=rr.offset,
                          ap=[rr.ap[0], [1, 4], [0, HD]])
            h = 2 * d + hh
            nc.vector.tensor_tensor(
                o_sb[:, t2 * 4:(t2 + 1) * 4, h * 64:(h + 1) * 64],
                o_ps[:, :, 0:HD], rrv, op=ALU.mult)

        # ================= proj + ReduceScatter ===========================
        def proj_rs(t2):
            tsl = slice(t2 * 512, (t2 + 1) * 512)
            for tt in range(4):
                for d in range(DT):
                    t_ps = ppK.tile([128, 128], BF16, tag="k",
                                    name=f"tp{t2}{tt}{d}")
                    nc.tensor.transpose(
                        t_ps, o_sb[:, t2 * 4 + tt, d * 128:(d + 1) * 128],
                        ident)
                    nc.vector.tensor_copy(
                        ot_sb[:, d,
                              t2 * 512 + tt * 128:t2 * 512 + (tt + 1) * 128],
                        t_ps)
            for c in range(CT):
                csl = slice(c * 128, (c + 1) * 128)
                p_ps = ppK.tile([128, 512], F32, tag="k", name=f"pj{c}{t2}")
                for p in range(2):
                    if p == 0:
                        rhs = ot_sb[:, 0:2, tsl]
                    else:
                        rb = ot_sb[:, 2, tsl]
                        rhs = bass.AP(tensor=rb.tensor, offset=rb.offset,
                                      ap=[rb.ap[0], [0, 2], rb.ap[1]])
                    nc.tensor.matmul(p_ps, wp8[:, p, :, csl], rhs,
                                     start=(p == 0), stop=(p == 1),
                                     perf_mode=DRM)
                nc.vector.tensor_copy(x1_sb[:, c, tsl], p_ps)
            for blk in range(2):
                nc.gpsimd.dma_start(
                    cc_in[t2][blk].rearrange("(a p) t -> p a t", p=128),
                    x1_sb[:, :, t2 * 512 + blk * 256:t2 * 512 + (blk + 1) * 256])
            nc.gpsimd.collective_compute(
                "ReduceScatter", ALU.add, ins=[cc_in[t2]], outs=[cc_rs[t2]],
                replica_groups=groups)

        # ================= schedule: attention phase ======================
        make_q(0)
        make_kv(0)
        attn_one(0, 0, 0)
        attn_one(0, 1, 0)
        make_q(1)
        make_kv(1)
        attn_one(1, 0, 0)
        attn_one(1, 1, 0)
        make_q(2)
        make_kv(2)
        attn_one(2, 0, 0)
        attn_one(2, 1, 0)
        proj_rs(0)

        # ================= post-RS: residual + LN2 + MLP ==================
        h8s = []

        def mlp_q(q):
            qsl = slice(q * 256, (q + 1) * 256)
            rsx = upool.tile([128, CT, 256], FP8, tag="rsx", name=f"rsx{q}")
            nc.gpsimd.dma_start(rsx, cc_rs[q].rearrange("(a p) t -> p a t",
                                                        p=128))
            for c in range(CT):
                nc.vector.affine_then_add(x1h[:, c, qsl], rsx[:, c, :],
                                          xqh_sb[:, c, qsl],
                                          scale=INV_SW2,
                                          bias=pb_sb[:, c:c + 1])
            # LN2 stats (bf16 ones-matmuls)
            x2h = upool.tile([128, CT, 256], FP8, tag="x2h", name=f"x2h{q}")
            for c in range(CT):
                nc.gpsimd.tensor_tensor(x2h[:, c, :], x1h[:, c, qsl],
                                        x1h[:, c, qsl], op=ALU.mult)
            s2_ps = ppK.tile([1, 256], F32, tag="k", name=f"s2{q}")
            for c in range(CT):
                nc.tensor.matmul(s2_ps, ones_bf, x1h[:, c, qsl],
                                 start=(c == 0), stop=(c == CT - 1))
            mu2 = sm2.tile([1, 256], F32, tag="st2", name=f"mu2{q}")
            nc.vector.tensor_scalar_mul(mu2, s2_ps, 1.0 / DIM)
            mu2bf = sm2.tile([1, 256], BF16, tag="st2", name=f"mu2bf{q}")
            nc.vector.tensor_copy(mu2bf, mu2)
            mu2_b = bc_pool.tile([128, 256], BF16, tag="bc2", name=f"mu2b{q}")
            nc.gpsimd.partition_broadcast(mu2_b, mu2bf)
            q2_ps = ppK.tile([1, 256], F32, tag="k", name=f"q2{q}")
            for c in range(CT):
                nc.tensor.matmul(q2_ps, ones_bf, x2h[:, c, :],
                                 start=(c == 0), stop=(c == CT - 1))
            m22 = sm2.tile([1, 256], F32, tag="st2", name=f"m22{q}")
            nc.vector.tensor_tensor(m22, mu2, mu2, op=ALU.mult)
            var2 = sm2.tile([1, 256], F32, tag="st2", name=f"var2{q}")
            nc.vector.scalar_tensor_tensor(var2, q2_ps, 1.0 / DIM, m22,
                                           op0=ALU.mult, op1=ALU.subtract)
            lnv2 = sm2.tile([1, 256], F32, tag="st2", name=f"lnv2{q}")
            nc.scalar.activation(lnv2, var2, AF.Ln, bias=eps_t[:1, :],
                                 scale=1.0)
            rs2 = sm2.tile([1, 256], F32, tag="st2", name=f"rs2{q}")
            nc.scalar.activation(rs2, lnv2, AF.Exp, scale=-0.5)
            rs2bf = sm2.tile([1, 256], BF16, tag="st2", name=f"rs2bf{q}")
            nc.vector.tensor_copy(rs2bf, rs2)
            rs2_b = bc_pool.tile([128, 256], BF16, tag="bc2", name=f"rs2b{q}")
            nc.gpsimd.partition_broadcast(rs2_b, rs2bf)

            def bview(t):
                a = t[:, :]
                return bass.AP(tensor=a.tensor, offset=a.offset,
                               ap=[a.ap[0], [0, CT], a.ap[1]])

            tx = upool.tile([128, CT, 256], BF16, tag="u", name=f"tx{q}")
            nc.vector.tensor_tensor(tx, x1h[:, :, qsl], bview(mu2_b),
                                    op=ALU.subtract)
            xn2b = upool.tile([128, CT, 256], BF16, tag="u", name=f"xn2b{q}")
            nc.vector.tensor_tensor(xn2b, tx, bview(rs2_b), op=ALU.mult)
            xn2 = upool.tile([128, CP, 2, 256], FP8, tag="xn2", name=f"xn2{q}")
            nc.gpsimd.tensor_copy(xn2.rearrange("r p s t -> r (p s) t"), xn2b)
            xn2r = upool.tile([128, CP, 2, 256], FP8, tag="xn2r",
                              name=f"xn2r{q}")
            nc.gpsimd.tensor_tensor(xn2r.rearrange("r p s t -> r (p s) t"),
                                    xn2b,
                                    xn2.rearrange("r p s t -> r (p s) t"),
                                    op=ALU.subtract)

            # reuse dead slots: q0 -> xq8 (dead after make_q/LN1),
            # q1 -> o_sb (dead after proj(1) transposes)
            if q == 0:
                h8 = big.tile([128, HP, 2, 256], FP8, tag="xq8", name="h8q0")
            else:
                h8 = med.tile([128, HP, 2, 256], FP8, tag="osb", name="h8q1")
            h8s.append(h8)
            for mg in range(HP // 2):
                h_ps = ppS.tile([128, 4, 256], F32, tag="s", name=f"h{q}{mg}")
                for mi in range(4):
                    m = 4 * mg + mi
                    msl = slice(m * 128, (m + 1) * 128)
                    nc.tensor.matmul(h_ps[:, mi, :], b1p8[:, m, :, :], ones8r,
                                     start=True, stop=False, perf_mode=DRM)
                    for p in range(CP):
                        nc.tensor.matmul(h_ps[:, mi, :], wm18[:, p, :, msl],
                                         xn2[:, p, :, :], start=False,
                                         stop=False, perf_mode=DRM)
                    for p in range(CP):
                        nc.tensor.matmul(h_ps[:, mi, :], wm18r[:, p, :, msl],
                                         xn2[:, p, :, :], start=False,
                                         stop=False, perf_mode=DRM)
                    for p in range(CP):
                        nc.tensor.matmul(h_ps[:, mi, :], wm18[:, p, :, msl],
                                         xn2r[:, p, :, :], start=False,
                                         stop=(p == CP - 1), perf_mode=DRM)
                nc.scalar.activation(
                    h8[:, 2 * mg:2 * mg + 2, :, :],
                    h_ps.rearrange("r (a s) t -> r a s t", a=2), AF.Gelu,
                    scale=INV_SW)

        def mlp2_q(q):
            qsl = slice(q * 256, (q + 1) * 256)
            h8 = h8s[q]
            for cp in range(CP):
                o2_ps = ppK.tile([128, 2, 256], F32, tag="k",
                                 name=f"o2{q}{cp}")
                for ci in range(2):
                    c = 2 * cp + ci
                    csl = slice(c * 128, (c + 1) * 128)
                    wm2c = upool.tile([128, HP, 2, 128], FP8, tag="wm2c",
                                      name=f"wm2c{q}{cp}{ci}")
                    nc.sync.dma_start(wm2c, wm28_d[:, :, :, csl])
                    wm2rc = upool.tile([128, HP, 2, 128], FP8, tag="wm2rc",
                                       name=f"wm2rc{q}{cp}{ci}")
                    nc.sync.dma_start(wm2rc, wm28r_d[:, :, :, csl])
                    for hp in range(HP):
                        nc.tensor.matmul(o2_ps[:, ci, :],
                                         wm2c[:, hp, :, :],
                                         h8[:, hp, :, :], start=(hp == 0),
                                         stop=False, perf_mode=DRM)
                    for hp in range(HP):
                        nc.tensor.matmul(o2_ps[:, ci, :],
                                         wm2rc[:, hp, :, :],
                                         h8[:, hp, :, :], start=False,
                                         stop=(hp == HP - 1), perf_mode=DRM)
                for ci in range(2):
                    c = 2 * cp + ci
                    csl = slice(c * 128, (c + 1) * 128)
                    fin = outp.tile([128, 256], F32, tag="outp",
                                    name=f"fin{q}{cp}{ci}")
                    nc.vector.affine_then_add(fin, o2_ps[:, ci, :],
                                              x1h[:, c, qsl], scale=INV_SW,
                                              bias=b2_sb[:, c:c + 1])
                    nc.sync.dma_start(out_d[csl, qsl], fin)

        attn_one(0, 0, 1)
        attn_one(0, 1, 1)
        attn_one(1, 0, 1)
        mlp_q(0)
        attn_one(1, 1, 1)
        attn_one(2, 0, 1)
        mlp2_q(0)
        attn_one(2, 1, 1)
        proj_rs(1)
        mlp_q(1)
        mlp2_q(1)

    nc.compile()
    return nc


def _prep_inputs(inputs):
    """Build the 8 per-core in_maps from the full-size inputs."""
    f8 = np.float64
    xq = np.asarray(inputs["xq"], np.float32)
    xkv = np.asarray(inputs["xkv"], np.float32)
    n1w = np.asarray(inputs["norm1_w"], f8); n1b = np.asarray(inputs["norm1_b"], f8)
    kv_w = np.asarray(inputs["kv_w"], f8); kv_b = np.asarray(inputs["kv_b"], f8)
    q_w = np.asarray(inputs["q_w"], f8); q_b = np.asarray(inputs["q_b"], f8)
    p_w = np.asarray(inputs["proj_w"], f8); p_b = np.asarray(inputs["proj_b"], f8)
    n2w = np.asarray(inputs["norm2_w"], f8); n2b = np.asarray(inputs["norm2_b"], f8)
    w1 = np.asarray(inputs["mlp_w1"], f8); b1 = np.asarray(inputs["mlp_b1"], f8)
    w2 = np.asarray(inputs["mlp_w2"], f8); b2 = np.asarray(inputs["mlp_b2"], f8)

    def cpair(mat, ncols):
        """[768, ncols] -> [128, 3, 2, ncols] channel-pair layout."""
        return np.ascontiguousarray(
            mat.reshape(CP, 2, 128, ncols).transpose(2, 0, 1, 3))

    wq_f = n1w[:, None] * q_w                                # [768, 768]
    qb_f = (q_b + n1b @ q_w) * SW                            # [768]
    kvw = kv_w.reshape(DIM, 2, NH, HD)
    kvb = kv_b.reshape(2, NH, HD)
    # v-bias folded into proj bias: sum over ALL heads
    pb_f = p_b + kvb[1].reshape(NH * HD) @ p_w               # [768]
    wm1_f = n2w[:, None] * w1
    b1_f = b1 + n2b @ w1
    wm18 = cpair((SW * wm1_f), MLPD).astype(F8)
    wm18r = cpair(
        SW * wm1_f - cpair(SW * wm1_f, MLPD).astype(F8).astype(np.float64)
        .transpose(1, 2, 0, 3).reshape(DIM, MLPD), MLPD).astype(F8)
    wm28 = np.ascontiguousarray(
        (SW * w2).reshape(HP, 2, 128, DIM).transpose(2, 0, 1, 3)).astype(F8)
    wm28r = np.ascontiguousarray(
        ((SW * w2) - (SW * w2).astype(F8).astype(np.float64))
        .reshape(HP, 2, 128, DIM).transpose(2, 0, 1, 3)).astype(F8)
    b1p8 = np.zeros((1, HT, 2, 128), F8)
    b1p8[0, :, 0, :] = (SW * b1_f).reshape(HT, 128).astype(F8)

    maps = []
    for core in range(NCORES):
        b, half = divmod(core, 2)
        hs = slice(half * NHL, (half + 1) * NHL)
        tidx = np.r_[half * 256:(half + 1) * 256,
                     512 + half * 256:512 + (half + 1) * 256]
        xqT = np.ascontiguousarray(xq[b].reshape(TQ, DIM).T)
        xkvT = np.ascontiguousarray(xkv[b].reshape(TKV, DIM).T)
        wq_c = cpair(SW * np.ascontiguousarray(
            wq_f.reshape(DIM, NH, HD)[:, hs].reshape(DIM, DL)), DL).astype(F8)
        wp_loc = SW * p_w.reshape(NH, HD, DIM)[hs].reshape(DL, DIM)
        wp_pad = np.zeros((4, 128, DIM), f8)
        wp_pad[0:DT] = wp_loc.reshape(DT, 128, DIM)
        wp8 = np.ascontiguousarray(
            wp_pad.reshape(2, 2, 128, DIM).transpose(2, 0, 1, 3)).astype(F8)
        m = {
            "xq8": cpair(xqT, TQ).astype(F8),
            "xqh": np.ascontiguousarray(
                xqT[:, tidx].reshape(CT, 128, TH).transpose(1, 0, 2)
            ).astype(BF),
            "xkv8": cpair(xkvT, TKV).astype(F8),
            "wq8": wq_c,
            "wk8": cpair(SW * np.ascontiguousarray(
                kvw[:, 0, hs].reshape(DIM, DL)), DL).astype(F8),
            "wv8": cpair(SW * np.ascontiguousarray(
                kvw[:, 1, hs].reshape(DIM, DL)), DL).astype(F8),
            "wp8": wp8,
            "wm18": wm18,
            "wm18r": wm18r,
            "wm28": wm28,
            "wm28r": wm28r,
            "b1p8": b1p8,
            # sq: column sums of the fp8 wq (exact, f64), [DL]
            "sq": wq_c.astype(np.float64).sum(axis=(0, 1, 2)).astype(
                np.float32),
            "qb": np.ascontiguousarray(
                qb_f.reshape(NH, HD)[hs].reshape(DL)).astype(np.float32),
            "pb": pb_f.astype(np.float32),
            "b2": b2.astype(np.float32),
        }
        maps.append(m)
    return maps


def kernel(**inputs):
    if "nc" not in _CACHE:
        _CACHE["nc"] = _build_program()
    nc = _CACHE["nc"]
    maps = _prep_inputs(inputs)
    res = run_bass_kernel_spmd(nc, maps, core_ids=list(range(NCORES)))
    out = np.zeros((B, TQ, DIM), np.float32)
    for core in range(NCORES):
        b, half = divmod(core, 2)
        tidx = np.r_[half * 256:(half + 1) * 256,
                     512 + half * 256:512 + (half + 1) * 256]
        x2T = res.results[core]["out"]          # [768, 512]
        out[b, tidx, :] = x2T.T
    return out.reshape(B, 32, 32, DIM)


# revision 21
# speedup vs baseline: 1.0945x; 1.0945x over previous
"""CrossBlock (cross-attention transformer block) on 8 TRN2 NeuronCores.

Sharding: 4 batch elements x 2 cores each (tensor-parallel over heads).
Core c = 2*b + half handles batch b; half selects heads 6*half..6*half+5.

v2: fp8(e4m3) DoubleRow matmuls everywhere (2 k-tiles per instruction),
softmax exp split between the Scalar (ACT) engine (true exp) and the Vector
(DVE) engine (one-instruction cubic-poly exp via a custom DVE op), psum->sbuf
copies spread over ACT/DVE, sbuf-only elementwise work on GPSIMD (Pool).
LayerNorm rstd via exp(-0.5*ln(var+eps)) so the attention phase stays within
one ACT function table. Weights are host-scaled by SW=32 to keep fp8 operands
out of the subnormal range; the inverse scales fold into activation-scale /
affine ops. Biases that only shift logits uniformly per query (k bias) are
dropped; v bias is folded into the proj bias on the host.

Per-core flow:
  LN1 stats (fp8 ones-matmuls) -> Q/K/V projections (fp8 DR) -> attention
  (S^T fp8 DR with a zero k-slot; exp ACT/DVE split; P@V fp8 DR with a ones
  column for the softmax denominator; normalize via tensor-tensor divide with
  a stride-0 denominator view) -> proj partial -> pairwise fp8 ReduceScatter
  (token split) -> residual + LN2 + MLP (fp8 DR, gelu on ACT, bias via a
  K=1 matmul) -> f32 output.
"""

import numpy as np
import ml_dtypes
from contextlib import ExitStack

import concourse.bass as bass
import concourse.tile as tile
from concourse import bacc, mybir
from concourse.bass_utils import run_bass_kernel_spmd
from concourse.masks import make_identity

F32 = mybir.dt.float32
BF16 = mybir.dt.bfloat16
FP8 = mybir.dt.float8e4
AF = mybir.ActivationFunctionType
ALU = mybir.AluOpType
DRM = mybir.MatmulPerfMode.DoubleRow
BF = ml_dtypes.bfloat16
F8 = ml_dtypes.float8_e4m3

DIM = 768
NH = 12
HD = 64
MLPD = 3072
EPS = 1e-5
B = 4
TQ = 1024          # query tokens per batch element
TKV = 4096         # kv tokens per batch element
NHL = NH // 2      # heads per core (6)
DL = NHL * HD      # local head cols (384)
TH = TQ // 2       # token half for the MLP stage (512)
CT = DIM // 128    # channel tiles (6)
CP = CT // 2       # channel pairs (3)
DT = DL // 128     # local head-pair groups (3)
HT = MLPD // 128   # hidden tiles (24)
HP = HT // 2       # hidden pairs (12)
KTT = TKV // 128   # kv token tiles (32)
NCORES = 8

SW = 32.0                      # host-side fp8 weight scale
ASC = HD ** -0.5               # attention scale (1/8)
ALPHA = ASC / (SW * SW)        # fold of attn scale + q/k weight scales
INV_SW = 1.0 / SW
INV_SW2 = 1.0 / (SW * SW)

# exp split: one ACT (true exp) per EXP_RATIO kt-groups, rest DVE poly.
EXP_ACT_OF = 8     # of every 16 groups, this many go to ACT

_CACHE = {}

# ---------------------------------------------------------------------------
# custom DVE op: one-instruction cubic exp approximation
#   f(s) = C0*s^3 + C1*s^2 + imm2*s + 1  (= Taylor of exp(imm2*s) when
#   C0=imm2^3/6, C1=imm2^2/2). The constant term 1 is exact, which keeps
#   softmax normalization consistent with the ACT-exp share.
# ---------------------------------------------------------------------------
import concourse.dve_ops as dve_ops
from concourse.dve_spec import Spec, Src0, C0, C1, C2, One, lower as dve_lower
from concourse.dve_uop import DveOpSpec


def _register_exp_poly():
    if hasattr(dve_ops, "_EXP_POLY3_OP"):
        return dve_ops._EXP_POLY3_OP
    body = ((Src0 * C0 + C1) * Src0 + C2) * Src0 + One
    spec = Spec(
        body=body,
        reference=lambda in0, in1, c0, c1, c2: (
            ((in0.astype(np.float32) * c0 + c1) * in0 + c2) * in0 + 1.0
        ),
    )
    name = "EXP_POLY3"
    opcode = dve_ops._CUSTOM_DVE_ROW_BASE + len(dve_ops.OPS)
    shas = {}
    for ver in ("v3", "v4"):
        s = DveOpSpec(name=name, opcode=opcode, uops=dve_lower(spec, ver=ver),
                      rd1_en=False)
        shas[ver] = s.sha(ver)
    op = dve_ops.DveOp(name, spec, subdim=False, uops_sha=shas)
    dve_ops.OPS.append(op)
    dve_ops._SUB_OPCODE_FOR_NAME[name] = opcode
    dve_ops.CUSTOM_DVE_SPECS[name] = spec
    dve_ops._EXP_POLY3_OP = op
    return op


EXP_POLY3 = _register_exp_poly()
P3_C0 = ALPHA ** 3 / 6.0
P3_C1 = ALPHA ** 2 / 2.0


def _build_program():
    nc = bacc.Bacc("TRN2", target_bir_lowering=False, debug=False,
                   num_devices=NCORES)

    din = {}

    def inp(name, shape, dt):
        din[name] = nc.dram_tensor(name, list(shape), dt,
                                   kind="ExternalInput").ap()
        return din[name]

    xq8_d = inp("xq8", (128, CP, 2, TQ), FP8)
    xqh_d = inp("xqh", (128, CT, TH), BF16)
    xkv8_d = inp("xkv8", (128, CP, 2, TKV), FP8)
    wq8_d = inp("wq8", (128, CP, 2, DL), FP8)
    wk8_d = inp("wk8", (128, CP, 2, DL), FP8)
    wv8_d = inp("wv8", (128, CP, 2, DL), FP8)
    wp8_d = inp("wp8", (128, 2, 2, DIM), FP8)
    wm18_d = inp("wm18", (128, CP, 2, MLPD), FP8)
    wm28_d = inp("wm28", (128, HP, 2, DIM), FP8)
    b1p8_d = inp("b1p8", (1, HT, 2, 128), FP8)
    sq_d = inp("sq", (DL,), F32)
    qb_d = inp("qb", (DL,), F32)
    pb_d = inp("pb", (DIM,), F32)
    b2_d = inp("b2", (DIM,), F32)
    out_d = nc.dram_tensor("out", [DIM, TH], F32, kind="ExternalOutput").ap()

    cc_in = [nc.dram_tensor(f"cc_in{i}", [2, DIM, TH // 2], FP8).ap()
             for i in range(2)]
    cc_rs = [nc.dram_tensor(f"cc_rs{i}", [DIM, TH // 2], FP8).ap()
             for i in range(2)]
    groups = [[0, 1], [2, 3], [4, 5], [6, 7]]

    with tile.TileContext(nc) as tc, ExitStack() as ctx:
        # ---- pools ----
        const = ctx.enter_context(tc.tile_pool(name="const", bufs=1))
        big = ctx.enter_context(tc.tile_pool(name="big", bufs=1))
        kvp = ctx.enter_context(tc.tile_pool(name="kvp", bufs=1))
        sexp_pool = ctx.enter_context(tc.tile_pool(name="sexp", bufs=9))
        med = ctx.enter_context(tc.tile_pool(name="med", bufs=1))
        sm = ctx.enter_context(tc.tile_pool(name="sm", bufs=2))
        sm2 = ctx.enter_context(tc.tile_pool(name="sm2", bufs=3))
        upool = ctx.enter_context(tc.tile_pool(name="upool", bufs=2))
        outp = ctx.enter_context(tc.tile_pool(name="outp", bufs=2))
        bc_pool = ctx.enter_context(tc.tile_pool(name="bc", bufs=2))

        # PSUM: ppS 2x2 banks (S pairs / MLP h), ppo 2x1 (PV accum),
        # ppK 2x1 (K/V/Q/proj/MLP2/stats)
        ppS = ctx.enter_context(tc.tile_pool(name="ppS", bufs=3, space="PSUM"))
        ppo = ctx.enter_context(tc.tile_pool(name="ppo", bufs=1, space="PSUM"))
        ppK = ctx.enter_context(tc.tile_pool(name="ppK", bufs=1, space="PSUM"))

        # ---- constants ----
        ones_bf = const.tile([128, 1], BF16)
        nc.vector.memset(ones_bf, 1.0)
        ones8_t = const.tile([128, 2, 16], FP8)
        nc.vector.memset(ones8_t, 1.0)
        ones8 = ones8_t[:, :, 0:1]
        ones8r = const.tile([1, 2, 256], FP8)
        nc.vector.memset(ones8r, 1.0)
        ident = const.tile([128, 128], BF16)
        make_identity(nc, ident)
        eps_t = const.tile([1, 1], F32)
        nc.vector.memset(eps_t, EPS)

        # ---- resident inputs / weights ----
        xq8 = big.tile([128, CP, 2, TQ], FP8, tag="xq8")
        nc.sync.dma_start(xq8, xq8_d)
        xkv8 = big.tile([128, CP, 2, TKV], FP8, tag="xkv8")
        nc.sync.dma_start(xkv8[:, :, :, 0:2048],
                          xkv8_d[:, :, :, 0:2048])
        nc.sync.dma_start(xkv8[:, :, :, 2048:TKV],
                          xkv8_d[:, :, :, 2048:TKV])
        wq8 = const.tile([128, CP, 2, DL], FP8)
        nc.sync.dma_start(wq8, wq8_d)
        wk8 = const.tile([128, CP, 2, DL], FP8)
        nc.sync.dma_start(wk8, wk8_d)
        wv8 = const.tile([128, CP, 2, DL], FP8)
        nc.sync.dma_start(wv8, wv8_d)
        xqh_sb = big.tile([128, CT, TH], BF16, tag="xqh")
        nc.sync.dma_start(xqh_sb, xqh_d)
        wp8 = const.tile([128, 2, 2, DIM], FP8)
        nc.sync.dma_start(wp8, wp8_d)
        wm18 = big.tile([128, CP, 2, MLPD], FP8, tag="wm1")
        nc.sync.dma_start(wm18, wm18_d)
        wm18r_d = nc.dram_tensor(
            "wm18r", [128, CP, 2, MLPD], FP8, kind="ExternalInput").ap()
        wm28r_d = nc.dram_tensor(
            "wm28r", [128, HP, 2, DIM], FP8, kind="ExternalInput").ap()
        b1p8 = const.tile([1, HT, 2, 128], FP8)
        nc.sync.dma_start(b1p8, b1p8_d)
        sq_sb = const.tile([128, DT], F32)
        nc.sync.dma_start(sq_sb, sq_d.rearrange("(a p) -> p a", p=128))
        qb_sb = const.tile([128, DT], F32)
        nc.sync.dma_start(qb_sb, qb_d.rearrange("(a p) -> p a", p=128))
        pb_sb = const.tile([128, CT], F32)
        nc.sync.dma_start(pb_sb, pb_d.rearrange("(a p) -> p a", p=128))
        b2_sb = const.tile([128, CT], F32)
        nc.sync.dma_start(b2_sb, b2_d.rearrange("(a p) -> p a", p=128))

        # ---- persistent attention tiles ----
        # K^T per d-group: [128(dl of 2 heads), TKV] fp8
        kt_sbs = [kvp.tile([128, TKV], FP8, tag=f"kt{d}", name=f"kt{d}")
                  for d in range(DT)]
        # V per d-group: [128(kt), group(16), slot(2), head(2), HD+1] fp8
        v_sbs = [kvp.tile([128, KTT // 2, 2, 2, HD + 1], FP8, tag=f"v{d}",
                          name=f"v{d}")
                 for d in range(DT)]
        # Q^T per d-group: [128(dl), slot(2), TQ] fp8, slot1 = zeros
        qt_sbs = [kvp.tile([128, 2, TQ], FP8, tag=f"qt{d}", name=f"qt{d}")
                  for d in range(DT)]
        for d in range(DT):
            nc.gpsimd.memset(qt_sbs[d][:, 1, :], 0.0)
            nc.gpsimd.memset(v_sbs[d][:, :, :, :, HD:HD + 1], 1.0)

        o_sb = med.tile([128, 8, DL], BF16, tag="osb")       # normalized O
        ot_sb = med.tile([128, DT, TQ], FP8, tag="ot")       # O^T for proj
        x1_sb = med.tile([128, CT, TQ], FP8, tag="x1")       # proj partial
        x1h = med.tile([128, CT, TH], BF16, tag="x1h")       # post-RS resid

        # ================= LN1 stats (from fp8 xq) ========================
        xsq8 = big.tile([128, CP, 2, TQ], FP8, tag="wm1r", name="xsq8")
        for p in range(CP):
            nc.gpsimd.tensor_tensor(xsq8[:, p], xq8[:, p], xq8[:, p],
                                    op=ALU.mult)

        mu_row = sm.tile([1, TQ], BF16, tag="st1")
        rs_row = sm.tile([1, TQ], BF16, tag="st1")
        for t2 in range(2):
            tsl = slice(t2 * 512, (t2 + 1) * 512)
            s_ps = ppK.tile([1, 512], F32, tag="k", name=f"sps{t2}")
            for p in range(CP):
                nc.tensor.matmul(s_ps, ones8, xq8[:, p, :, tsl],
                                 start=(p == 0), stop=(p == CP - 1),
                                 perf_mode=DRM)
            nc.vector.tensor_scalar_mul(mu_row[:, tsl], s_ps, 1.0 / DIM)
            q_ps = ppK.tile([1, 512], F32, tag="k", name=f"qps{t2}")
            for p in range(CP):
                nc.tensor.matmul(q_ps, ones8, xsq8[:, p, :, tsl],
                                 start=(p == 0), stop=(p == CP - 1),
                                 perf_mode=DRM)
            m2 = sm2.tile([1, 512], F32, tag="st2", name=f"m2{t2}")
            nc.vector.tensor_tensor(m2, mu_row[:, tsl], mu_row[:, tsl],
                                    op=ALU.mult)
            var = sm2.tile([1, 512], F32, tag="st2", name=f"var{t2}")
            nc.vector.scalar_tensor_tensor(var, q_ps, 1.0 / DIM, m2,
                                           op0=ALU.mult, op1=ALU.subtract)
            lnv = sm2.tile([1, 512], F32, tag="st2", name=f"lnv{t2}")
            nc.scalar.activation(lnv, var, AF.Ln, bias=eps_t[:1, :], scale=1.0)
            nc.scalar.activation(rs_row[:, tsl], lnv, AF.Exp, scale=-0.5)
        mu_b = bc_pool.tile([128, TQ], BF16, tag="bc")
        nc.gpsimd.partition_broadcast(mu_b, mu_row)
        rs_b = bc_pool.tile([128, TQ], BF16, tag="bc")
        nc.gpsimd.partition_broadcast(rs_b, rs_row)
        # wm18r reuses xsq8's slot (tag wm1r); DMA lands after stats read it
        wm18r = big.tile([128, CP, 2, MLPD], FP8, tag="wm1r", name="wm18r")
        nc.sync.dma_start(wm18r, wm18r_d)

        # ================= projections ====================================
        def make_q(d):
            dsl = slice(d * 128, (d + 1) * 128)
            qt = qt_sbs[d]
            for t2 in range(2):
                tsl = slice(t2 * 512, (t2 + 1) * 512)
                y_ps = ppK.tile([128, 512], F32, tag="k", name=f"y{d}{t2}")
                for p in range(CP):
                    nc.tensor.matmul(y_ps, wq8[:, p, :, dsl],
                                     xq8[:, p, :, tsl], start=(p == 0),
                                     stop=(p == CP - 1), perf_mode=DRM)
                u = upool.tile([128, 512], F32, tag="u", name=f"u{d}{t2}")
                nc.vector.scalar_tensor_tensor(u, mu_b[:, tsl],
                                               sq_sb[:, d:d + 1], y_ps,
                                               op0=ALU.mult, op1=ALU.subtract)
                v2 = upool.tile([128, 512], F32, tag="u", name=f"v{d}{t2}")
                nc.gpsimd.tensor_tensor(v2, u, rs_b[:, tsl], op=ALU.mult)
                nc.vector.tensor_scalar(qt[:, 0, tsl], v2, scalar1=-1.0,
                                        op0=ALU.mult,
                                        scalar2=qb_sb[:, d:d + 1],
                                        op1=ALU.add)

        def make_kv(d):
            dsl = slice(d * 128, (d + 1) * 128)
            kt, v_sb = kt_sbs[d], v_sbs[d]
            for ch in range(TKV // 512):
                ksl = slice(ch * 512, (ch + 1) * 512)
                k_ps = ppK.tile([128, 512], F32, tag="k", name=f"k{d}{ch}")
                for p in range(CP):
                    nc.tensor.matmul(k_ps, wk8[:, p, :, dsl],
                                     xkv8[:, p, :, ksl], start=(p == 0),
                                     stop=(p == CP - 1), perf_mode=DRM)
                nc.scalar.copy(kt[:, ksl], k_ps)
                v_ps = ppK.tile([128, 4, 128], F32, tag="k", name=f"vp{d}{ch}")
                for j in range(4):
                    ktt = ch * 4 + j
                    ktsl = slice(ktt * 128, (ktt + 1) * 128)
                    for p in range(CP):
                        nc.tensor.matmul(v_ps[:, j, :],
                                         xkv8[:, p, :, ktsl],
                                         wv8[:, p, :, dsl], start=(p == 0),
                                         stop=(p == CP - 1), perf_mode=DRM)
                # [128,4,128] -> v_sb[:, 2ch:2ch+2, :, :, 0:HD]
                nc.scalar.copy(
                    v_sb[:, 2 * ch:2 * ch + 2, :, :, 0:HD],
                    v_ps.rearrange("q (g s) (h x) -> q g s h x", g=2, h=2))

        # ================= attention ======================================
        def attn_one(d, hh, t2):
            qt, kt, v_sb = qt_sbs[d], kt_sbs[d], v_sbs[d]
            tsl = slice(t2 * 512, (t2 + 1) * 512)
            rsl = slice(hh * 64, hh * 64 + 64)
            o_ps = ppo.tile([128, 4, HD + 1], F32, tag="o",
                            name=f"ops{d}{hh}{t2}")
            q_dr = qt[rsl, :, tsl]
            NW = 8                      # groups per wave
            for w in range(KTT // 2 // NW):
                sexps = []
                for gg in range(NW):
                    g = w * NW + gg
                    s_ps = ppS.tile([128, 2, 512], F32, tag="s",
                                    name=f"s{d}{hh}{t2}{g}")
                    for i in range(2):
                        ktt = 2 * g + i
                        kbase = kt[rsl, ktt * 128:(ktt + 1) * 128]
                        k_dr = bass.AP(tensor=kbase.tensor,
                                       offset=kbase.offset,
                                       ap=[kbase.ap[0], [0, 2], kbase.ap[1]])
                        nc.tensor.matmul(s_ps[:, i, :], k_dr, q_dr,
                                         start=True, stop=True, perf_mode=DRM)
                    sexp = sexp_pool.tile([128, 2, 512], FP8, tag="se",
                                          name=f"se{d}{hh}{t2}{g}")
                    sexps.append(sexp)
                    if g % 2 == 0:
                        nc.scalar.activation(sexp, s_ps, AF.Exp, scale=ALPHA)
                    else:
                        nc.vector._custom_dve(EXP_POLY3, out=sexp, in0=s_ps,
                                              s0=P3_C0, s1=P3_C1, imm2=ALPHA)
                for gg in range(NW):
                    g = w * NW + gg
                    for tt in range(4):
                        nc.tensor.matmul(
                            o_ps[:, tt, :],
                            sexps[gg][:, :, tt * 128:(tt + 1) * 128],
                            v_sb[:, g, :, hh, :],
                            start=(g == 0), stop=(g == KTT // 2 - 1),
                            perf_mode=DRM)
            # normalize: o = o_raw * (1/denom), stride-0 reciprocal view
            rr = sm.tile([128, 4], F32, tag="rr", name=f"rr{d}{hh}{t2}")
            nc.vector.reciprocal(rr, o_ps[:, :, HD])
            rrv = bass.AP(tensor=rr.tensor, offset=rr.offset,
                          ap=[rr.ap[0], [1, 4], [0, HD]])
            h = 2 * d + hh
            nc.vector.tensor_tensor(
                o_sb[:, t2 * 4:(t2 + 1) * 4, h * 64:(h + 1) * 64],
                o_ps[:, :, 0:HD], rrv, op=ALU.mult)

        # ================= proj + ReduceScatter ===========================
        def proj_rs(t2):
            tsl = slice(t2 * 512, (t2 + 1) * 512)
            for tt in range(4):
                for d in range(DT):
                    t_ps = ppK.tile([128, 128], BF16, tag="k",
                                    name=f"tp{t2}{tt}{d}")
                    nc.tensor.transpose(
                        t_ps, o_sb[:, t2 * 4 + tt, d * 128:(d + 1) * 128],
                        ident)
                    nc.vector.tensor_copy(
                        ot_sb[:, d,
                              t2 * 512 + tt * 128:t2 * 512 + (tt + 1) * 128],
                        t_ps)
            for c in range(CT):
                csl = slice(c * 128, (c + 1) * 128)
                p_ps = ppK.tile([128, 512], F32, tag="k", name=f"pj{c}{t2}")
                for p in range(2):
                    if p == 0:
                        rhs = ot_sb[:, 0:2, tsl]
                    else:
                        rb = ot_sb[:, 2, tsl]
                        rhs = bass.AP(tensor=rb.tensor, offset=rb.offset,
                                      ap=[rb.ap[0], [0, 2], rb.ap[1]])
                    nc.tensor.matmul(p_ps, wp8[:, p, :, csl], rhs,
                                     start=(p == 0), stop=(p == 1),
                                     perf_mode=DRM)
                nc.vector.tensor_copy(x1_sb[:, c, tsl], p_ps)
            for blk in range(2):
                nc.gpsimd.dma_start(
                    cc_in[t2][blk].rearrange("(a p) t -> p a t", p=128),
                    x1_sb[:, :, t2 * 512 + blk * 256:t2 * 512 + (blk + 1) * 256])
            nc.gpsimd.collective_compute(
                "ReduceScatter", ALU.add, ins=[cc_in[t2]], outs=[cc_rs[t2]],
                replica_groups=groups)

        # ================= schedule: attention phase ======================
        make_q(0)
        make_kv(0)
        attn_one(0, 0, 0)
        attn_one(0, 1, 0)
        make_q(1)
        make_kv(1)
        attn_one(1, 0, 0)
        attn_one(1, 1, 0)
        make_q(2)
        make_kv(2)
        # xkv8 is dead now; its slot hosts wm28 for the MLP tail
        wm28 = big.tile([128, HP, 2, DIM], FP8, tag="xkv8", name="wm28")
        nc.sync.dma_start(wm28, wm28_d)
        attn_one(2, 0, 0)
        attn_one(2, 1, 0)
        proj_rs(0)

        # ================= post-RS: residual + LN2 + MLP ==================
        h8s = []

        def mlp_q(q):
            qsl = slice(q * 256, (q + 1) * 256)
            rsx = upool.tile([128, CT, 256], FP8, tag="rsx", name=f"rsx{q}")
            nc.gpsimd.dma_start(rsx, cc_rs[q].rearrange("(a p) t -> p a t",
                                                        p=128))
            for c in range(CT):
                nc.vector.affine_then_add(x1h[:, c, qsl], rsx[:, c, :],
                                          xqh_sb[:, c, qsl],
                                          scale=INV_SW2,
                                          bias=pb_sb[:, c:c + 1])
            # LN2 stats (bf16 ones-matmuls)
            x2h = upool.tile([128, CT, 256], FP8, tag="x2h", name=f"x2h{q}")
            for c in range(CT):
                nc.gpsimd.tensor_tensor(x2h[:, c, :], x1h[:, c, qsl],
                                        x1h[:, c, qsl], op=ALU.mult)
            s2_ps = ppK.tile([1, 256], F32, tag="k", name=f"s2{q}")
            for c in range(CT):
                nc.tensor.matmul(s2_ps, ones_bf, x1h[:, c, qsl],
                                 start=(c == 0), stop=(c == CT - 1))
            mu2 = sm2.tile([1, 256], F32, tag="st2", name=f"mu2{q}")
            nc.vector.tensor_scalar_mul(mu2, s2_ps, 1.0 / DIM)
            mu2bf = sm2.tile([1, 256], BF16, tag="st2", name=f"mu2bf{q}")
            nc.vector.tensor_copy(mu2bf, mu2)
            mu2_b = bc_pool.tile([128, 256], BF16, tag="bc2", name=f"mu2b{q}")
            nc.gpsimd.partition_broadcast(mu2_b, mu2bf)
            q2_ps = ppK.tile([1, 256], F32, tag="k", name=f"q2{q}")
            for c in range(CT):
                nc.tensor.matmul(q2_ps, ones_bf, x2h[:, c, :],
                                 start=(c == 0), stop=(c == CT - 1))
            m22 = sm2.tile([1, 256], F32, tag="st2", name=f"m22{q}")
            nc.vector.tensor_tensor(m22, mu2, mu2, op=ALU.mult)
            var2 = sm2.tile([1, 256], F32, tag="st2", name=f"var2{q}")
            nc.vector.scalar_tensor_tensor(var2, q2_ps, 1.0 / DIM, m22,
                                           op0=ALU.mult, op1=ALU.subtract)
            lnv2 = sm2.tile([1, 256], F32, tag="st2", name=f"lnv2{q}")
            nc.scalar.activation(lnv2, var2, AF.Ln, bias=eps_t[:1, :],
                                 scale=1.0)
            rs2 = sm2.tile([1, 256], F32, tag="st2", name=f"rs2{q}")
            nc.scalar.activation(rs2, lnv2, AF.Exp, scale=-0.5)
            rs2bf = sm2.tile([1, 256], BF16, tag="st2", name=f"rs2bf{q}")
            nc.vector.tensor_copy(rs2bf, rs2)
            rs2_b = bc_pool.tile([128, 256], BF16, tag="bc2", name=f"rs2b{q}")
            nc.gpsimd.partition_broadcast(rs2_b, rs2bf)

            def bview(t):
                a = t[:, :]
                return bass.AP(tensor=a.tensor, offset=a.offset,
                               ap=[a.ap[0], [0, CT], a.ap[1]])

            tx = upool.tile([128, CT, 256], BF16, tag="u", name=f"tx{q}")
            nc.vector.tensor_tensor(tx, x1h[:, :, qsl], bview(mu2_b),
                                    op=ALU.subtract)
            xn2b = upool.tile([128, CT, 256], BF16, tag="u", name=f"xn2b{q}")
            nc.vector.tensor_tensor(xn2b, tx, bview(rs2_b), op=ALU.mult)
            xn2 = upool.tile([128, CP, 2, 256], FP8, tag="xn2", name=f"xn2{q}")
            nc.gpsimd.tensor_copy(xn2.rearrange("r p s t -> r (p s) t"), xn2b)
            xn2r = upool.tile([128, CP, 2, 256], FP8, tag="xn2r",
                              name=f"xn2r{q}")
            nc.gpsimd.tensor_tensor(xn2r.rearrange("r p s t -> r (p s) t"),
                                    xn2b,
                                    xn2.rearrange("r p s t -> r (p s) t"),
                                    op=ALU.subtract)

            # reuse dead slots: q0 -> xq8 (dead after make_q/LN1),
            # q1 -> o_sb (dead after proj(1) transposes)
            if q == 0:
                h8 = big.tile([128, HP, 2, 256], FP8, tag="xq8", name="h8q0")
            else:
                h8 = med.tile([128, HP, 2, 256], FP8, tag="osb", name="h8q1")
            h8s.append(h8)
            for mg in range(HP // 2):
                h_ps = ppS.tile([128, 4, 256], F32, tag="s", name=f"h{q}{mg}")
                for mi in range(4):
                    m = 4 * mg + mi
                    msl = slice(m * 128, (m + 1) * 128)
                    nc.tensor.matmul(h_ps[:, mi, :], b1p8[:, m, :, :], ones8r,
                                     start=True, stop=False, perf_mode=DRM)
                    for p in range(CP):
                        nc.tensor.matmul(h_ps[:, mi, :], wm18[:, p, :, msl],
                                         xn2[:, p, :, :], start=False,
                                         stop=False, perf_mode=DRM)
                    for p in range(CP):
                        nc.tensor.matmul(h_ps[:, mi, :], wm18r[:, p, :, msl],
                                         xn2[:, p, :, :], start=False,
                                         stop=False, perf_mode=DRM)
                    for p in range(CP):
                        nc.tensor.matmul(h_ps[:, mi, :], wm18[:, p, :, msl],
                                         xn2r[:, p, :, :], start=False,
                                         stop=(p == CP - 1), perf_mode=DRM)
                nc.scalar.activation(
                    h8[:, 2 * mg:2 * mg + 2, :, :],
                    h_ps.rearrange("r (a s) t -> r a s t", a=2), AF.Gelu,
                    scale=INV_SW)

        def mlp2_q(q):
            qsl = slice(q * 256, (q + 1) * 256)
            h8 = h8s[q]
            for cp in range(CP):
                o2_ps = ppK.tile([128, 2, 256], F32, tag="k",
                                 name=f"o2{q}{cp}")
                for ci in range(2):
                    c = 2 * cp + ci
                    csl = slice(c * 128, (c + 1) * 128)
                    wm2rc = upool.tile([128, HP, 2, 128], FP8, tag="wm2rc",
                                       name=f"wm2rc{q}{cp}{ci}")
                    nc.sync.dma_start(wm2rc, wm28r_d[:, :, :, csl])
                    for hp in range(HP):
                        nc.tensor.matmul(o2_ps[:, ci, :],
                                         wm28[:, hp, :, csl],
                                         h8[:, hp, :, :], start=(hp == 0),
                                         stop=False, perf_mode=DRM)
                    for hp in range(HP):
                        nc.tensor.matmul(o2_ps[:, ci, :],
                                         wm2rc[:, hp, :, :],
                                         h8[:, hp, :, :], start=False,
                                         stop=(hp == HP - 1), perf_mode=DRM)
                for ci in range(2):
                    c = 2 * cp + ci
                    csl = slice(c * 128, (c + 1) * 128)
                    fin = outp.tile([128, 256], F32, tag="outp",
                                    name=f"fin{q}{cp}{ci}")
                    nc.vector.affine_then_add(fin, o2_ps[:, ci, :],
                                              x1h[:, c, qsl], scale=INV_SW,
                                              bias=b2_sb[:, c:c + 1])
                    nc.sync.dma_start(out_d[csl, qsl], fin)

        attn_one(0, 0, 1)
        attn_one(0, 1, 1)
        attn_one(1, 0, 1)
        attn_one(1, 1, 1)
        attn_one(2, 0, 1)
        attn_one(2, 1, 1)
        proj_rs(1)
        mlp_q(0)
        mlp2_q(0)
        mlp_q(1)
        mlp2_q(1)

    nc.compile()
    return nc


def _prep_inputs(inputs):
    """Build the 8 per-core in_maps from the full-size inputs."""
    f8 = np.float64
    xq = np.asarray(inputs["xq"], np.float32)
    xkv = np.asarray(inputs["xkv"], np.float32)
    n1w = np.asarray(inputs["norm1_w"], f8); n1b = np.asarray(inputs["norm1_b"], f8)
    kv_w = np.asarray(inputs["kv_w"], f8); kv_b = np.asarray(inputs["kv_b"], f8)
    q_w = np.asarray(inputs["q_w"], f8); q_b = np.asarray(inputs["q_b"], f8)
    p_w = np.asarray(inputs["proj_w"], f8); p_b = np.asarray(inputs["proj_b"], f8)
    n2w = np.asarray(inputs["norm2_w"], f8); n2b = np.asarray(inputs["norm2_b"], f8)
    w1 = np.asarray(inputs["mlp_w1"], f8); b1 = np.asarray(inputs["mlp_b1"], f8)
    w2 = np.asarray(inputs["mlp_w2"], f8); b2 = np.asarray(inputs["mlp_b2"], f8)

    def cpair(mat, ncols):
        """[768, ncols] -> [128, 3, 2, ncols] channel-pair layout."""
        return np.ascontiguousarray(
            mat.reshape(CP, 2, 128, ncols).transpose(2, 0, 1, 3))

    wq_f = n1w[:, None] * q_w                                # [768, 768]
    qb_f = (q_b + n1b @ q_w) * SW                            # [768]
    kvw = kv_w.reshape(DIM, 2, NH, HD)
    kvb = kv_b.reshape(2, NH, HD)
    # v-bias folded into proj bias: sum over ALL heads
    pb_f = p_b + kvb[1].reshape(NH * HD) @ p_w               # [768]
    wm1_f = n2w[:, None] * w1
    b1_f = b1 + n2b @ w1
    wm18 = cpair((SW * wm1_f), MLPD).astype(F8)
    wm18r = cpair(
        SW * wm1_f - cpair(SW * wm1_f, MLPD).astype(F8).astype(np.float64)
        .transpose(1, 2, 0, 3).reshape(DIM, MLPD), MLPD).astype(F8)
    wm28 = np.ascontiguousarray(
        (SW * w2).reshape(HP, 2, 128, DIM).transpose(2, 0, 1, 3)).astype(F8)
    wm28r = np.ascontiguousarray(
        ((SW * w2) - (SW * w2).astype(F8).astype(np.float64))
        .reshape(HP, 2, 128, DIM).transpose(2, 0, 1, 3)).astype(F8)
    b1p8 = np.zeros((1, HT, 2, 128), F8)
    b1p8[0, :, 0, :] = (SW * b1_f).reshape(HT, 128).astype(F8)

    maps = []
    for core in range(NCORES):
        b, half = divmod(core, 2)
        hs = slice(half * NHL, (half + 1) * NHL)
        tidx = np.r_[half * 256:(half + 1) * 256,
                     512 + half * 256:512 + (half + 1) * 256]
        xqT = np.ascontiguousarray(xq[b].reshape(TQ, DIM).T)
        xkvT = np.ascontiguousarray(xkv[b].reshape(TKV, DIM).T)
        wq_c = cpair(SW * np.ascontiguousarray(
            wq_f.reshape(DIM, NH, HD)[:, hs].reshape(DIM, DL)), DL).astype(F8)
        wp_loc = SW * p_w.reshape(NH, HD, DIM)[hs].reshape(DL, DIM)
        wp_pad = np.zeros((4, 128, DIM), f8)
        wp_pad[0:DT] = wp_loc.reshape(DT, 128, DIM)
        wp8 = np.ascontiguousarray(
            wp_pad.reshape(2, 2, 128, DIM).transpose(2, 0, 1, 3)).astype(F8)
        m = {
            "xq8": cpair(xqT, TQ).astype(F8),
            "xqh": np.ascontiguousarray(
                xqT[:, tidx].reshape(CT, 128, TH).transpose(1, 0, 2)
            ).astype(BF),
            "xkv8": cpair(xkvT, TKV).astype(F8),
            "wq8": wq_c,
            "wk8": cpair(SW * np.ascontiguousarray(
                kvw[:, 0, hs].reshape(DIM, DL)), DL).astype(F8),
            "wv8": cpair(SW * np.ascontiguousarray(
                kvw[:, 1, hs].reshape(DIM, DL)), DL).astype(F8),
            "wp8": wp8,
            "wm18": wm18,
            "wm18r": wm18r,
            "wm28": wm28,
            "wm28r": wm28r,
            "b1p8": b1p8,
            # sq: column sums of the fp8 wq (exact, f64), [DL]
            "sq": wq_c.astype(np.float64).sum(axis=(0, 1, 2)).astype(
                np.float32),
            "qb": np.ascontiguousarray(
                qb_f.reshape(NH, HD)[hs].reshape(DL)).astype(np.float32),
            "pb": pb_f.astype(np.float32),
            "b2": b2.astype(np.float32),
        }
        maps.append(m)
    return maps


def kernel(**inputs):
    if "nc" not in _CACHE:
        _CACHE["nc"] = _build_program()
    nc = _CACHE["nc"]
    maps = _prep_inputs(inputs)
    res = run_bass_kernel_spmd(nc, maps, core_ids=list(range(NCORES)))
    out = np.zeros((B, TQ, DIM), np.float32)
    for core in range(NCORES):
        b, half = divmod(core, 2)
        tidx = np.r_[half * 256:(half + 1) * 256,
                     512 + half * 256:512 + (half + 1) * 256]
        x2T = res.results[core]["out"]          # [768, 512]
        out[b, tidx, :] = x2T.T
    return out.reshape(B, 32, 32, DIM)


# revision 22
# speedup vs baseline: 1.1447x; 1.0459x over previous
"""CrossBlock (cross-attention transformer block) on 8 TRN2 NeuronCores.

Sharding: 4 batch elements x 2 cores each (tensor-parallel over heads).
Core c = 2*b + half handles batch b; half selects heads 6*half..6*half+5.

v2: fp8(e4m3) DoubleRow matmuls everywhere (2 k-tiles per instruction),
softmax exp split between the Scalar (ACT) engine (true exp) and the Vector
(DVE) engine (one-instruction cubic-poly exp via a custom DVE op), psum->sbuf
copies spread over ACT/DVE, sbuf-only elementwise work on GPSIMD (Pool).
LayerNorm rstd via exp(-0.5*ln(var+eps)) so the attention phase stays within
one ACT function table. Weights are host-scaled by SW=32 to keep fp8 operands
out of the subnormal range; the inverse scales fold into activation-scale /
affine ops. Biases that only shift logits uniformly per query (k bias) are
dropped; v bias is folded into the proj bias on the host.

Per-core flow:
  LN1 stats (fp8 ones-matmuls) -> Q/K/V projections (fp8 DR) -> attention
  (S^T fp8 DR with a zero k-slot; exp ACT/DVE split; P@V fp8 DR with a ones
  column for the softmax denominator; normalize via tensor-tensor divide with
  a stride-0 denominator view) -> proj partial -> pairwise fp8 ReduceScatter
  (token split) -> residual + LN2 + MLP (fp8 DR, gelu on ACT, bias via a
  K=1 matmul) -> f32 output.
"""

import numpy as np
import ml_dtypes
from contextlib import ExitStack

import concourse.bass as bass
import concourse.tile as tile
from concourse import bacc, mybir
from concourse.bass_utils import run_bass_kernel_spmd
from concourse.masks import make_identity

F32 = mybir.dt.float32
BF16 = mybir.dt.bfloat16
FP8 = mybir.dt.float8e4
AF = mybir.ActivationFunctionType
ALU = mybir.AluOpType
DRM = mybir.MatmulPerfMode.DoubleRow
BF = ml_dtypes.bfloat16
F8 = ml_dtypes.float8_e4m3

DIM = 768
NH = 12
HD = 64
MLPD = 3072
EPS = 1e-5
B = 4
TQ = 1024          # query tokens per batch element
TKV = 4096         # kv tokens per batch element
NHL = NH // 2      # heads per core (6)
DL = NHL * HD      # local head cols (384)
TH = TQ // 2       # token half for the MLP stage (512)
CT = DIM // 128    # channel tiles (6)
CP = CT // 2       # channel pairs (3)
DT = DL // 128     # local head-pair groups (3)
HT = MLPD // 128   # hidden tiles (24)
HP = HT // 2       # hidden pairs (12)
KTT = TKV // 128   # kv token tiles (32)
NCORES = 8

SW = 32.0                      # host-side fp8 weight scale
ASC = HD ** -0.5               # attention scale (1/8)
ALPHA = ASC / (SW * SW)        # fold of attn scale + q/k weight scales
INV_SW = 1.0 / SW
INV_SW2 = 1.0 / (SW * SW)

# exp split: one ACT (true exp) per EXP_RATIO kt-groups, rest DVE poly.
EXP_ACT_OF = 8     # of every 16 groups, this many go to ACT

_CACHE = {}

# ---------------------------------------------------------------------------
# custom DVE op: one-instruction cubic exp approximation
#   f(s) = C0*s^3 + C1*s^2 + imm2*s + 1  (= Taylor of exp(imm2*s) when
#   C0=imm2^3/6, C1=imm2^2/2). The constant term 1 is exact, which keeps
#   softmax normalization consistent with the ACT-exp share.
# ---------------------------------------------------------------------------
import concourse.dve_ops as dve_ops
from concourse.dve_spec import Spec, Src0, C0, C1, C2, One, lower as dve_lower
from concourse.dve_uop import DveOpSpec


def _register_exp_poly():
    if hasattr(dve_ops, "_EXP_POLY3_OP"):
        return dve_ops._EXP_POLY3_OP
    body = ((Src0 * C0 + C1) * Src0 + C2) * Src0 + One
    spec = Spec(
        body=body,
        reference=lambda in0, in1, c0, c1, c2: (
            ((in0.astype(np.float32) * c0 + c1) * in0 + c2) * in0 + 1.0
        ),
    )
    name = "EXP_POLY3"
    opcode = dve_ops._CUSTOM_DVE_ROW_BASE + len(dve_ops.OPS)
    shas = {}
    for ver in ("v3", "v4"):
        s = DveOpSpec(name=name, opcode=opcode, uops=dve_lower(spec, ver=ver),
                      rd1_en=False)
        shas[ver] = s.sha(ver)
    op = dve_ops.DveOp(name, spec, subdim=False, uops_sha=shas)
    dve_ops.OPS.append(op)
    dve_ops._SUB_OPCODE_FOR_NAME[name] = opcode
    dve_ops.CUSTOM_DVE_SPECS[name] = spec
    dve_ops._EXP_POLY3_OP = op
    return op


EXP_POLY3 = _register_exp_poly()
P3_C0 = ALPHA ** 3 / 6.0
P3_C1 = ALPHA ** 2 / 2.0


def _build_program():
    nc = bacc.Bacc("TRN2", target_bir_lowering=False, debug=False,
                   num_devices=NCORES)

    din = {}

    def inp(name, shape, dt):
        din[name] = nc.dram_tensor(name, list(shape), dt,
                                   kind="ExternalInput").ap()
        return din[name]

    xq8_d = inp("xq8", (128, CP, 2, TQ), FP8)
    xqh_d = inp("xqh", (128, CT, TH), BF16)
    xkv8_d = inp("xkv8", (128, CP, 2, TKV), FP8)
    wq8_d = inp("wq8", (128, CP, 2, DL), FP8)
    wk8_d = inp("wk8", (128, CP, 2, DL), FP8)
    wv8_d = inp("wv8", (128, CP, 2, DL), FP8)
    wp8_d = inp("wp8", (128, 2, 2, DIM), FP8)
    wm18_d = inp("wm18", (128, CP, 2, MLPD), FP8)
    wm28_d = inp("wm28", (128, HP, 2, DIM), FP8)
    b1p8_d = inp("b1p8", (1, HT, 2, 128), FP8)
    sq_d = inp("sq", (DL,), F32)
    qb_d = inp("qb", (DL,), F32)
    pb_d = inp("pb", (DIM,), F32)
    b2_d = inp("b2", (DIM,), F32)
    out_d = nc.dram_tensor("out", [DIM, TH], F32, kind="ExternalOutput").ap()

    cc_in = [nc.dram_tensor(f"cc_in{i}", [2, DIM, TH // 2], FP8).ap()
             for i in range(2)]
    cc_rs = [nc.dram_tensor(f"cc_rs{i}", [DIM, TH // 2], FP8).ap()
             for i in range(2)]
    groups = [[0, 1], [2, 3], [4, 5], [6, 7]]

    with tile.TileContext(nc) as tc, ExitStack() as ctx:
        # ---- pools ----
        const = ctx.enter_context(tc.tile_pool(name="const", bufs=1))
        big = ctx.enter_context(tc.tile_pool(name="big", bufs=1))
        kvp = ctx.enter_context(tc.tile_pool(name="kvp", bufs=1))
        sexp_pool = ctx.enter_context(tc.tile_pool(name="sexp", bufs=9))
        med = ctx.enter_context(tc.tile_pool(name="med", bufs=1))
        sm = ctx.enter_context(tc.tile_pool(name="sm", bufs=2))
        sm2 = ctx.enter_context(tc.tile_pool(name="sm2", bufs=3))
        upool = ctx.enter_context(tc.tile_pool(name="upool", bufs=2))
        outp = ctx.enter_context(tc.tile_pool(name="outp", bufs=2))
        bc_pool = ctx.enter_context(tc.tile_pool(name="bc", bufs=2))

        # PSUM: ppS 2x2 banks (S pairs / MLP h), ppo 2x1 (PV accum),
        # ppK 2x1 (K/V/Q/proj/MLP2/stats)
        ppS = ctx.enter_context(tc.tile_pool(name="ppS", bufs=3, space="PSUM"))
        ppo = ctx.enter_context(tc.tile_pool(name="ppo", bufs=1, space="PSUM"))
        ppK = ctx.enter_context(tc.tile_pool(name="ppK", bufs=1, space="PSUM"))

        # ---- constants ----
        ones_bf = const.tile([128, 1], BF16)
        nc.vector.memset(ones_bf, 1.0)
        ones8_t = const.tile([128, 2, 16], FP8)
        nc.vector.memset(ones8_t, 1.0)
        ones8 = ones8_t[:, :, 0:1]
        ones8r = const.tile([1, 2, 256], FP8)
        nc.vector.memset(ones8r, 1.0)
        ident = const.tile([128, 128], BF16)
        make_identity(nc, ident)
        eps_t = const.tile([1, 1], F32)
        nc.vector.memset(eps_t, EPS)

        # ---- resident inputs / weights ----
        xq8 = big.tile([128, CP, 2, TQ], FP8, tag="xq8")
        nc.sync.dma_start(xq8, xq8_d)
        xkv8 = big.tile([128, CP, 2, TKV], FP8, tag="xkv8")
        nc.sync.dma_start(xkv8[:, :, :, 0:2048],
                          xkv8_d[:, :, :, 0:2048])
        nc.sync.dma_start(xkv8[:, :, :, 2048:TKV],
                          xkv8_d[:, :, :, 2048:TKV])
        wq8 = const.tile([128, CP, 2, DL], FP8)
        nc.sync.dma_start(wq8, wq8_d)
        wk8 = const.tile([128, CP, 2, DL], FP8)
        nc.sync.dma_start(wk8, wk8_d)
        wv8 = const.tile([128, CP, 2, DL], FP8)
        nc.sync.dma_start(wv8, wv8_d)
        xqh_sb = big.tile([128, CT, TH], BF16, tag="xqh")
        nc.sync.dma_start(xqh_sb, xqh_d)
        wp8 = const.tile([128, 2, 2, DIM], FP8)
        nc.sync.dma_start(wp8, wp8_d)
        wm18 = big.tile([128, CP, 2, MLPD], FP8, tag="wm1")
        nc.sync.dma_start(wm18, wm18_d)
        wm18r_d = nc.dram_tensor(
            "wm18r", [128, CP, 2, MLPD], FP8, kind="ExternalInput").ap()
        wm28r_d = nc.dram_tensor(
            "wm28r", [128, HP, 2, DIM], FP8, kind="ExternalInput").ap()
        b1p8 = const.tile([1, HT, 2, 128], FP8)
        nc.sync.dma_start(b1p8, b1p8_d)
        sq_sb = const.tile([128, DT], F32)
        nc.sync.dma_start(sq_sb, sq_d.rearrange("(a p) -> p a", p=128))
        qb_sb = const.tile([128, DT], F32)
        nc.sync.dma_start(qb_sb, qb_d.rearrange("(a p) -> p a", p=128))
        pb_sb = const.tile([128, CT], F32)
        nc.sync.dma_start(pb_sb, pb_d.rearrange("(a p) -> p a", p=128))
        b2_sb = const.tile([128, CT], F32)
        nc.sync.dma_start(b2_sb, b2_d.rearrange("(a p) -> p a", p=128))

        # ---- persistent attention tiles ----
        # K^T per d-group: [128(dl of 2 heads), TKV] fp8
        kt_sbs = [kvp.tile([128, TKV], FP8, tag=f"kt{d}", name=f"kt{d}")
                  for d in range(DT)]
        # V per d-group: [128(kt), group(16), slot(2), head(2), HD+1] fp8
        v_sbs = [kvp.tile([128, KTT // 2, 2, 2, HD + 1], FP8, tag=f"v{d}",
                          name=f"v{d}")
                 for d in range(DT)]
        # Q^T per d-group: [128(dl), slot(2), TQ] fp8, slot1 = zeros
        qt_sbs = [kvp.tile([128, 2, TQ], FP8, tag=f"qt{d}", name=f"qt{d}")
                  for d in range(DT)]
        for d in range(DT):
            nc.gpsimd.memset(qt_sbs[d][:, 1, :], 0.0)
            nc.gpsimd.memset(v_sbs[d][:, :, :, :, HD:HD + 1], 1.0)

        o_sb = med.tile([128, 8, DL], BF16, tag="osb")       # normalized O
        ot_sb = med.tile([128, DT, TQ], FP8, tag="ot")       # O^T for proj
        x1_sb = med.tile([128, CT, TQ], FP8, tag="x1")       # proj partial
        x1h = med.tile([128, CT, TH], BF16, tag="x1h")       # post-RS resid

        # ================= LN1 stats (from fp8 xq) ========================
        xsq8 = big.tile([128, CP, 2, TQ], FP8, tag="wm1r", name="xsq8")
        for p in range(CP):
            nc.gpsimd.tensor_tensor(xsq8[:, p], xq8[:, p], xq8[:, p],
                                    op=ALU.mult)

        mu_row = sm.tile([1, TQ], BF16, tag="st1")
        rs_row = sm.tile([1, TQ], BF16, tag="st1")
        for t2 in range(2):
            tsl = slice(t2 * 512, (t2 + 1) * 512)
            s_ps = ppK.tile([1, 512], F32, tag="k", name=f"sps{t2}")
            for p in range(CP):
                nc.tensor.matmul(s_ps, ones8, xq8[:, p, :, tsl],
                                 start=(p == 0), stop=(p == CP - 1),
                                 perf_mode=DRM)
            nc.vector.tensor_scalar_mul(mu_row[:, tsl], s_ps, 1.0 / DIM)
            q_ps = ppK.tile([1, 512], F32, tag="k", name=f"qps{t2}")
            for p in range(CP):
                nc.tensor.matmul(q_ps, ones8, xsq8[:, p, :, tsl],
                                 start=(p == 0), stop=(p == CP - 1),
                                 perf_mode=DRM)
            m2 = sm2.tile([1, 512], F32, tag="st2", name=f"m2{t2}")
            nc.vector.tensor_tensor(m2, mu_row[:, tsl], mu_row[:, tsl],
                                    op=ALU.mult)
            var = sm2.tile([1, 512], F32, tag="st2", name=f"var{t2}")
            nc.vector.scalar_tensor_tensor(var, q_ps, 1.0 / DIM, m2,
                                           op0=ALU.mult, op1=ALU.subtract)
            lnv = sm2.tile([1, 512], F32, tag="st2", name=f"lnv{t2}")
            nc.scalar.activation(lnv, var, AF.Ln, bias=eps_t[:1, :], scale=1.0)
            nc.scalar.activation(rs_row[:, tsl], lnv, AF.Exp, scale=-0.5)
        mu_b = bc_pool.tile([128, TQ], BF16, tag="bc")
        nc.gpsimd.partition_broadcast(mu_b, mu_row)
        rs_b = bc_pool.tile([128, TQ], BF16, tag="bc")
        nc.gpsimd.partition_broadcast(rs_b, rs_row)
        # wm18r reuses xsq8's slot (tag wm1r); DMA lands after stats read it
        wm18r = big.tile([128, CP, 2, MLPD], FP8, tag="wm1r", name="wm18r")
        nc.sync.dma_start(wm18r, wm18r_d)

        # ================= projections ====================================
        def make_q(d):
            dsl = slice(d * 128, (d + 1) * 128)
            qt = qt_sbs[d]
            for t2 in range(2):
                tsl = slice(t2 * 512, (t2 + 1) * 512)
                y_ps = ppK.tile([128, 512], F32, tag="k", name=f"y{d}{t2}")
                for p in range(CP):
                    nc.tensor.matmul(y_ps, wq8[:, p, :, dsl],
                                     xq8[:, p, :, tsl], start=(p == 0),
                                     stop=(p == CP - 1), perf_mode=DRM)
                u = upool.tile([128, 512], F32, tag="u", name=f"u{d}{t2}")
                nc.vector.scalar_tensor_tensor(u, mu_b[:, tsl],
                                               sq_sb[:, d:d + 1], y_ps,
                                               op0=ALU.mult, op1=ALU.subtract)
                v2 = upool.tile([128, 512], F32, tag="u", name=f"v{d}{t2}")
                nc.gpsimd.tensor_tensor(v2, u, rs_b[:, tsl], op=ALU.mult)
                nc.vector.tensor_scalar(qt[:, 0, tsl], v2, scalar1=-1.0,
                                        op0=ALU.mult,
                                        scalar2=qb_sb[:, d:d + 1],
                                        op1=ALU.add)

        def make_kv(d):
            dsl = slice(d * 128, (d + 1) * 128)
            kt, v_sb = kt_sbs[d], v_sbs[d]
            for ch in range(TKV // 512):
                ksl = slice(ch * 512, (ch + 1) * 512)
                k_ps = ppK.tile([128, 512], F32, tag="k", name=f"k{d}{ch}")
                for p in range(CP):
                    nc.tensor.matmul(k_ps, wk8[:, p, :, dsl],
                                     xkv8[:, p, :, ksl], start=(p == 0),
                                     stop=(p == CP - 1), perf_mode=DRM)
                nc.scalar.copy(kt[:, ksl], k_ps)
                v_ps = ppK.tile([128, 4, 128], F32, tag="k", name=f"vp{d}{ch}")
                for j in range(4):
                    ktt = ch * 4 + j
                    ktsl = slice(ktt * 128, (ktt + 1) * 128)
                    for p in range(CP):
                        nc.tensor.matmul(v_ps[:, j, :],
                                         xkv8[:, p, :, ktsl],
                                         wv8[:, p, :, dsl], start=(p == 0),
                                         stop=(p == CP - 1), perf_mode=DRM)
                # [128,4,128] -> v_sb[:, 2ch:2ch+2, :, :, 0:HD]
                nc.scalar.copy(
                    v_sb[:, 2 * ch:2 * ch + 2, :, :, 0:HD],
                    v_ps.rearrange("q (g s) (h x) -> q g s h x", g=2, h=2))

        # ================= attention ======================================
        def attn_one(d, hh, t2):
            qt, kt, v_sb = qt_sbs[d], kt_sbs[d], v_sbs[d]
            tsl = slice(t2 * 512, (t2 + 1) * 512)
            rsl = slice(hh * 64, hh * 64 + 64)
            o_ps = ppo.tile([128, 4, HD + 1], F32, tag="o",
                            name=f"ops{d}{hh}{t2}")
            q_dr = qt[rsl, :, tsl]
            NW = 8                      # groups per wave
            for w in range(KTT // 2 // NW):
                sexps = []
                for gg in range(NW):
                    g = w * NW + gg
                    s_ps = ppS.tile([128, 2, 512], F32, tag="s",
                                    name=f"s{d}{hh}{t2}{g}")
                    for i in range(2):
                        ktt = 2 * g + i
                        kbase = kt[rsl, ktt * 128:(ktt + 1) * 128]
                        k_dr = bass.AP(tensor=kbase.tensor,
                                       offset=kbase.offset,
                                       ap=[kbase.ap[0], [0, 2], kbase.ap[1]])
                        nc.tensor.matmul(s_ps[:, i, :], k_dr, q_dr,
                                         start=True, stop=True, perf_mode=DRM)
                    sexp = sexp_pool.tile([128, 2, 512], FP8, tag="se",
                                          name=f"se{d}{hh}{t2}{g}")
                    sexps.append(sexp)
                    if g % 2 == 0:
                        nc.scalar.activation(sexp, s_ps, AF.Exp, scale=ALPHA)
                    else:
                        nc.vector._custom_dve(EXP_POLY3, out=sexp, in0=s_ps,
                                              s0=P3_C0, s1=P3_C1, imm2=ALPHA)
                for gg in range(NW):
                    g = w * NW + gg
                    for tt in range(4):
                        nc.tensor.matmul(
                            o_ps[:, tt, :],
                            sexps[gg][:, :, tt * 128:(tt + 1) * 128],
                            v_sb[:, g, :, hh, :],
                            start=(g == 0), stop=(g == KTT // 2 - 1),
                            perf_mode=DRM)
            # normalize: o = o_raw * (1/denom), stride-0 reciprocal view
            rr = sm.tile([128, 4], F32, tag="rr", name=f"rr{d}{hh}{t2}")
            nc.vector.reciprocal(rr, o_ps[:, :, HD])
            rrv = bass.AP(tensor=rr.tensor, offset=rr.offset,
                          ap=[rr.ap[0], [1, 4], [0, HD]])
            h = 2 * d + hh
            nc.vector.tensor_tensor(
                o_sb[:, t2 * 4:(t2 + 1) * 4, h * 64:(h + 1) * 64],
                o_ps[:, :, 0:HD], rrv, op=ALU.mult)

        # ================= proj + ReduceScatter ===========================
        def proj_rs(t2):
            tsl = slice(t2 * 512, (t2 + 1) * 512)
            for tt in range(4):
                for d in range(DT):
                    t_ps = ppK.tile([128, 128], BF16, tag="k",
                                    name=f"tp{t2}{tt}{d}")
                    nc.tensor.transpose(
                        t_ps, o_sb[:, t2 * 4 + tt, d * 128:(d + 1) * 128],
                        ident)
                    nc.vector.tensor_copy(
                        ot_sb[:, d,
                              t2 * 512 + tt * 128:t2 * 512 + (tt + 1) * 128],
                        t_ps)
            for c in range(CT):
                csl = slice(c * 128, (c + 1) * 128)
                p_ps = ppK.tile([128, 512], F32, tag="k", name=f"pj{c}{t2}")
                for p in range(2):
                    if p == 0:
                        rhs = ot_sb[:, 0:2, tsl]
                    else:
                        rb = ot_sb[:, 2, tsl]
                        rhs = bass.AP(tensor=rb.tensor, offset=rb.offset,
                                      ap=[rb.ap[0], [0, 2], rb.ap[1]])
                    nc.tensor.matmul(p_ps, wp8[:, p, :, csl], rhs,
                                     start=(p == 0), stop=(p == 1),
                                     perf_mode=DRM)
                nc.vector.tensor_copy(x1_sb[:, c, tsl], p_ps)
            for blk in range(2):
                nc.gpsimd.dma_start(
                    cc_in[t2][blk].rearrange("(a p) t -> p a t", p=128),
                    x1_sb[:, :, t2 * 512 + blk * 256:t2 * 512 + (blk + 1) * 256])
            nc.gpsimd.collective_compute(
                "ReduceScatter", ALU.add, ins=[cc_in[t2]], outs=[cc_rs[t2]],
                replica_groups=groups)

        # ================= schedule: attention phase ======================
        make_q(0)
        make_kv(0)
        attn_one(0, 0, 0)
        attn_one(0, 0, 1)
        make_q(1)
        make_kv(1)
        attn_one(0, 1, 0)
        attn_one(0, 1, 1)
        make_q(2)
        make_kv(2)
        # xkv8 is dead now; its slot hosts wm28 for the MLP tail
        wm28 = big.tile([128, HP, 2, DIM], FP8, tag="xkv8", name="wm28")
        nc.sync.dma_start(wm28, wm28_d)
        attn_one(1, 0, 0)
        attn_one(1, 1, 0)
        attn_one(2, 0, 0)
        attn_one(2, 1, 0)
        proj_rs(0)

        # ================= post-RS: residual + LN2 + MLP ==================
        h8s = []

        def mlp_q(q):
            qsl = slice(q * 256, (q + 1) * 256)
            rsx = upool.tile([128, CT, 256], FP8, tag="rsx", name=f"rsx{q}")
            nc.gpsimd.dma_start(rsx, cc_rs[q].rearrange("(a p) t -> p a t",
                                                        p=128))
            for c in range(CT):
                nc.vector.affine_then_add(x1h[:, c, qsl], rsx[:, c, :],
                                          xqh_sb[:, c, qsl],
                                          scale=INV_SW2,
                                          bias=pb_sb[:, c:c + 1])
            # LN2 stats (bf16 ones-matmuls)
            x2h = upool.tile([128, CT, 256], FP8, tag="x2h", name=f"x2h{q}")
            for c in range(CT):
                nc.gpsimd.tensor_tensor(x2h[:, c, :], x1h[:, c, qsl],
                                        x1h[:, c, qsl], op=ALU.mult)
            s2_ps = ppK.tile([1, 256], F32, tag="k", name=f"s2{q}")
            for c in range(CT):
                nc.tensor.matmul(s2_ps, ones_bf, x1h[:, c, qsl],
                                 start=(c == 0), stop=(c == CT - 1))
            mu2 = sm2.tile([1, 256], F32, tag="st2", name=f"mu2{q}")
            nc.vector.tensor_scalar_mul(mu2, s2_ps, 1.0 / DIM)
            mu2bf = sm2.tile([1, 256], BF16, tag="st2", name=f"mu2bf{q}")
            nc.vector.tensor_copy(mu2bf, mu2)
            mu2_b = bc_pool.tile([128, 256], BF16, tag="bc2", name=f"mu2b{q}")
            nc.gpsimd.partition_broadcast(mu2_b, mu2bf)
            q2_ps = ppK.tile([1, 256], F32, tag="k", name=f"q2{q}")
            for c in range(CT):
                nc.tensor.matmul(q2_ps, ones_bf, x2h[:, c, :],
                                 start=(c == 0), stop=(c == CT - 1))
            m22 = sm2.tile([1, 256], F32, tag="st2", name=f"m22{q}")
            nc.vector.tensor_tensor(m22, mu2, mu2, op=ALU.mult)
            var2 = sm2.tile([1, 256], F32, tag="st2", name=f"var2{q}")
            nc.vector.scalar_tensor_tensor(var2, q2_ps, 1.0 / DIM, m22,
                                           op0=ALU.mult, op1=ALU.subtract)
            lnv2 = sm2.tile([1, 256], F32, tag="st2", name=f"lnv2{q}")
            nc.scalar.activation(lnv2, var2, AF.Ln, bias=eps_t[:1, :],
                                 scale=1.0)
            rs2 = sm2.tile([1, 256], F32, tag="st2", name=f"rs2{q}")
            nc.scalar.activation(rs2, lnv2, AF.Exp, scale=-0.5)
            rs2bf = sm2.tile([1, 256], BF16, tag="st2", name=f"rs2bf{q}")
            nc.vector.tensor_copy(rs2bf, rs2)
            rs2_b = bc_pool.tile([128, 256], BF16, tag="bc2", name=f"rs2b{q}")
            nc.gpsimd.partition_broadcast(rs2_b, rs2bf)

            def bview(t):
                a = t[:, :]
                return bass.AP(tensor=a.tensor, offset=a.offset,
                               ap=[a.ap[0], [0, CT], a.ap[1]])

            tx = upool.tile([128, CT, 256], BF16, tag="u", name=f"tx{q}")
            nc.vector.tensor_tensor(tx, x1h[:, :, qsl], bview(mu2_b),
                                    op=ALU.subtract)
            xn2b = upool.tile([128, CT, 256], BF16, tag="u", name=f"xn2b{q}")
            nc.vector.tensor_tensor(xn2b, tx, bview(rs2_b), op=ALU.mult)
            xn2 = upool.tile([128, CP, 2, 256], FP8, tag="xn2", name=f"xn2{q}")
            nc.gpsimd.tensor_copy(xn2.rearrange("r p s t -> r (p s) t"), xn2b)
            xn2r = upool.tile([128, CP, 2, 256], FP8, tag="xn2r",
                              name=f"xn2r{q}")
            nc.gpsimd.tensor_tensor(xn2r.rearrange("r p s t -> r (p s) t"),
                                    xn2b,
                                    xn2.rearrange("r p s t -> r (p s) t"),
                                    op=ALU.subtract)

            # reuse dead slots: q0 -> xq8 (dead after make_q/LN1),
            # q1 -> o_sb (dead after proj(1) transposes)
            if q == 0:
                h8 = big.tile([128, HP, 2, 256], FP8, tag="xq8", name="h8q0")
            else:
                h8 = med.tile([128, HP, 2, 256], FP8, tag="osb", name="h8q1")
            h8s.append(h8)
            for mg in range(HP // 2):
                h_ps = ppS.tile([128, 4, 256], F32, tag="s", name=f"h{q}{mg}")
                for mi in range(4):
                    m = 4 * mg + mi
                    msl = slice(m * 128, (m + 1) * 128)
                    nc.tensor.matmul(h_ps[:, mi, :], b1p8[:, m, :, :], ones8r,
                                     start=True, stop=False, perf_mode=DRM)
                    for p in range(CP):
                        nc.tensor.matmul(h_ps[:, mi, :], wm18[:, p, :, msl],
                                         xn2[:, p, :, :], start=False,
                                         stop=False, perf_mode=DRM)
                    for p in range(CP):
                        nc.tensor.matmul(h_ps[:, mi, :], wm18r[:, p, :, msl],
                                         xn2[:, p, :, :], start=False,
                                         stop=False, perf_mode=DRM)
                    for p in range(CP):
                        nc.tensor.matmul(h_ps[:, mi, :], wm18[:, p, :, msl],
                                         xn2r[:, p, :, :], start=False,
                                         stop=(p == CP - 1), perf_mode=DRM)
                nc.scalar.activation(
                    h8[:, 2 * mg:2 * mg + 2, :, :],
                    h_ps.rearrange("r (a s) t -> r a s t", a=2), AF.Gelu,
                    scale=INV_SW)

        def mlp2_q(q):
            qsl = slice(q * 256, (q + 1) * 256)
            h8 = h8s[q]
            for cp in range(CP):
                o2_ps = ppK.tile([128, 2, 256], F32, tag="k",
                                 name=f"o2{q}{cp}")
                for ci in range(2):
                    c = 2 * cp + ci
                    csl = slice(c * 128, (c + 1) * 128)
                    wm2rc = upool.tile([128, HP, 2, 128], FP8, tag="wm2rc",
                                       name=f"wm2rc{q}{cp}{ci}")
                    nc.sync.dma_start(wm2rc, wm28r_d[:, :, :, csl])
                    for hp in range(HP):
                        nc.tensor.matmul(o2_ps[:, ci, :],
                                         wm28[:, hp, :, csl],
                                         h8[:, hp, :, :], start=(hp == 0),
                                         stop=False, perf_mode=DRM)
                    for hp in range(HP):
                        nc.tensor.matmul(o2_ps[:, ci, :],
                                         wm2rc[:, hp, :, :],
                                         h8[:, hp, :, :], start=False,
                                         stop=(hp == HP - 1), perf_mode=DRM)
                for ci in range(2):
                    c = 2 * cp + ci
                    csl = slice(c * 128, (c + 1) * 128)
                    fin = outp.tile([128, 256], F32, tag="outp",
                                    name=f"fin{q}{cp}{ci}")
                    nc.vector.affine_then_add(fin, o2_ps[:, ci, :],
                                              x1h[:, c, qsl], scale=INV_SW,
                                              bias=b2_sb[:, c:c + 1])
                    nc.sync.dma_start(out_d[csl, qsl], fin)

        attn_one(1, 0, 1)
        attn_one(1, 1, 1)
        attn_one(2, 0, 1)
        attn_one(2, 1, 1)
        proj_rs(1)
        mlp_q(0)
        mlp2_q(0)
        mlp_q(1)
        mlp2_q(1)

    nc.compile()
    return nc


def _prep_inputs(inputs):
    """Build the 8 per-core in_maps from the full-size inputs."""
    f8 = np.float64
    xq = np.asarray(inputs["xq"], np.float32)
    xkv = np.asarray(inputs["xkv"], np.float32)
    n1w = np.asarray(inputs["norm1_w"], f8); n1b = np.asarray(inputs["norm1_b"], f8)
    kv_w = np.asarray(inputs["kv_w"], f8); kv_b = np.asarray(inputs["kv_b"], f8)
    q_w = np.asarray(inputs["q_w"], f8); q_b = np.asarray(inputs["q_b"], f8)
    p_w = np.asarray(inputs["proj_w"], f8); p_b = np.asarray(inputs["proj_b"], f8)
    n2w = np.asarray(inputs["norm2_w"], f8); n2b = np.asarray(inputs["norm2_b"], f8)
    w1 = np.asarray(inputs["mlp_w1"], f8); b1 = np.asarray(inputs["mlp_b1"], f8)
    w2 = np.asarray(inputs["mlp_w2"], f8); b2 = np.asarray(inputs["mlp_b2"], f8)

    def cpair(mat, ncols):
        """[768, ncols] -> [128, 3, 2, ncols] channel-pair layout."""
        return np.ascontiguousarray(
            mat.reshape(CP, 2, 128, ncols).transpose(2, 0, 1, 3))

    wq_f = n1w[:, None] * q_w                                # [768, 768]
    qb_f = (q_b + n1b @ q_w) * SW                            # [768]
    kvw = kv_w.reshape(DIM, 2, NH, HD)
    kvb = kv_b.reshape(2, NH, HD)
    # v-bias folded into proj bias: sum over ALL heads
    pb_f = p_b + kvb[1].reshape(NH * HD) @ p_w               # [768]
    wm1_f = n2w[:, None] * w1
    b1_f = b1 + n2b @ w1
    wm18 = cpair((SW * wm1_f), MLPD).astype(F8)
    wm18r = cpair(
        SW * wm1_f - cpair(SW * wm1_f, MLPD).astype(F8).astype(np.float64)
        .transpose(1, 2, 0, 3).reshape(DIM, MLPD), MLPD).astype(F8)
    wm28 = np.ascontiguousarray(
        (SW * w2).reshape(HP, 2, 128, DIM).transpose(2, 0, 1, 3)).astype(F8)
    wm28r = np.ascontiguousarray(
        ((SW * w2) - (SW * w2).astype(F8).astype(np.float64))
        .reshape(HP, 2, 128, DIM).transpose(2, 0, 1, 3)).astype(F8)
    b1p8 = np.zeros((1, HT, 2, 128), F8)
    b1p8[0, :, 0, :] = (SW * b1_f).reshape(HT, 128).astype(F8)

    maps = []
    for core in range(NCORES):
        b, half = divmod(core, 2)
        hs = slice(half * NHL, (half + 1) * NHL)
        tidx = np.r_[half * 256:(half + 1) * 256,
                     512 + half * 256:512 + (half + 1) * 256]
        xqT = np.ascontiguousarray(xq[b].reshape(TQ, DIM).T)
        xkvT = np.ascontiguousarray(xkv[b].reshape(TKV, DIM).T)
        wq_c = cpair(SW * np.ascontiguousarray(
            wq_f.reshape(DIM, NH, HD)[:, hs].reshape(DIM, DL)), DL).astype(F8)
        wp_loc = SW * p_w.reshape(NH, HD, DIM)[hs].reshape(DL, DIM)
        wp_pad = np.zeros((4, 128, DIM), f8)
        wp_pad[0:DT] = wp_loc.reshape(DT, 128, DIM)
        wp8 = np.ascontiguousarray(
            wp_pad.reshape(2, 2, 128, DIM).transpose(2, 0, 1, 3)).astype(F8)
        m = {
            "xq8": cpair(xqT, TQ).astype(F8),
            "xqh": np.ascontiguousarray(
                xqT[:, tidx].reshape(CT, 128, TH).transpose(1, 0, 2)
            ).astype(BF),
            "xkv8": cpair(xkvT, TKV).astype(F8),
            "wq8": wq_c,
            "wk8": cpair(SW * np.ascontiguousarray(
                kvw[:, 0, hs].reshape(DIM, DL)), DL).astype(F8),
            "wv8": cpair(SW * np.ascontiguousarray(
                kvw[:, 1, hs].reshape(DIM, DL)), DL).astype(F8),
            "wp8": wp8,
            "wm18": wm18,
            "wm18r": wm18r,
            "wm28": wm28,
            "wm28r": wm28r,
            "b1p8": b1p8,
            # sq: column sums of the fp8 wq (exact, f64), [DL]
            "sq": wq_c.astype(np.float64).sum(axis=(0, 1, 2)).astype(
                np.float32),
            "qb": np.ascontiguousarray(
                qb_f.reshape(NH, HD)[hs].reshape(DL)).astype(np.float32),
            "pb": pb_f.astype(np.float32),
            "b2": b2.astype(np.float32),
        }
        maps.append(m)
    return maps


def kernel(**inputs):
    if "nc" not in _CACHE:
        _CACHE["nc"] = _build_program()
    nc = _CACHE["nc"]
    maps = _prep_inputs(inputs)
    res = run_bass_kernel_spmd(nc, maps, core_ids=list(range(NCORES)))
    out = np.zeros((B, TQ, DIM), np.float32)
    for core in range(NCORES):
        b, half = divmod(core, 2)
        tidx = np.r_[half * 256:(half + 1) * 256,
                     512 + half * 256:512 + (half + 1) * 256]
        x2T = res.results[core]["out"]          # [768, 512]
        out[b, tidx, :] = x2T.T
    return out.reshape(B, 32, 32, DIM)


# revision 24
# speedup vs baseline: 1.1512x; 1.0056x over previous
"""CrossBlock (cross-attention transformer block) on 8 TRN2 NeuronCores.

Sharding: 4 batch elements x 2 cores each (tensor-parallel over heads).
Core c = 2*b + half handles batch b; half selects heads 6*half..6*half+5.

v2: fp8(e4m3) DoubleRow matmuls everywhere (2 k-tiles per instruction),
softmax exp split between the Scalar (ACT) engine (true exp) and the Vector
(DVE) engine (one-instruction cubic-poly exp via a custom DVE op), psum->sbuf
copies spread over ACT/DVE, sbuf-only elementwise work on GPSIMD (Pool).
LayerNorm rstd via exp(-0.5*ln(var+eps)) so the attention phase stays within
one ACT function table. Weights are host-scaled by SW=32 to keep fp8 operands
out of the subnormal range; the inverse scales fold into activation-scale /
affine ops. Biases that only shift logits uniformly per query (k bias) are
dropped; v bias is folded into the proj bias on the host.

Per-core flow:
  LN1 stats (fp8 ones-matmuls) -> Q/K/V projections (fp8 DR) -> attention
  (S^T fp8 DR with a zero k-slot; exp ACT/DVE split; P@V fp8 DR with a ones
  column for the softmax denominator; normalize via tensor-tensor divide with
  a stride-0 denominator view) -> proj partial -> pairwise fp8 ReduceScatter
  (token split) -> residual + LN2 + MLP (fp8 DR, gelu on ACT, bias via a
  K=1 matmul) -> f32 output.
"""

import numpy as np
import ml_dtypes
from contextlib import ExitStack

import concourse.bass as bass
import concourse.tile as tile
from concourse import bacc, mybir
from concourse.bass_utils import run_bass_kernel_spmd
from concourse.masks import make_identity

F32 = mybir.dt.float32
BF16 = mybir.dt.bfloat16
FP8 = mybir.dt.float8e4
AF = mybir.ActivationFunctionType
ALU = mybir.AluOpType
DRM = mybir.MatmulPerfMode.DoubleRow
BF = ml_dtypes.bfloat16
F8 = ml_dtypes.float8_e4m3

DIM = 768
NH = 12
HD = 64
MLPD = 3072
EPS = 1e-5
B = 4
TQ = 1024          # query tokens per batch element
TKV = 4096         # kv tokens per batch element
NHL = NH // 2      # heads per core (6)
DL = NHL * HD      # local head cols (384)
TH = TQ // 2       # token half for the MLP stage (512)
CT = DIM // 128    # channel tiles (6)
CP = CT // 2       # channel pairs (3)
DT = DL // 128     # local head-pair groups (3)
HT = MLPD // 128   # hidden tiles (24)
HP = HT // 2       # hidden pairs (12)
KTT = TKV // 128   # kv token tiles (32)
NCORES = 8

SW = 32.0                      # host-side fp8 weight scale
ASC = HD ** -0.5               # attention scale (1/8)
ALPHA = ASC / (SW * SW)        # fold of attn scale + q/k weight scales
INV_SW = 1.0 / SW
INV_SW2 = 1.0 / (SW * SW)

# exp split: one ACT (true exp) per EXP_RATIO kt-groups, rest DVE poly.
EXP_ACT_OF = 8     # of every 16 groups, this many go to ACT

_CACHE = {}

# ---------------------------------------------------------------------------
# custom DVE op: one-instruction cubic exp approximation
#   f(s) = C0*s^3 + C1*s^2 + imm2*s + 1  (= Taylor of exp(imm2*s) when
#   C0=imm2^3/6, C1=imm2^2/2). The constant term 1 is exact, which keeps
#   softmax normalization consistent with the ACT-exp share.
# ---------------------------------------------------------------------------
import concourse.dve_ops as dve_ops
from concourse.dve_spec import Spec, Src0, C0, C1, C2, One, lower as dve_lower
from concourse.dve_uop import DveOpSpec


def _register_exp_poly():
    if hasattr(dve_ops, "_EXP_POLY3_OP"):
        return dve_ops._EXP_POLY3_OP
    body = ((Src0 * C0 + C1) * Src0 + C2) * Src0 + One
    spec = Spec(
        body=body,
        reference=lambda in0, in1, c0, c1, c2: (
            ((in0.astype(np.float32) * c0 + c1) * in0 + c2) * in0 + 1.0
        ),
    )
    name = "EXP_POLY3"
    opcode = dve_ops._CUSTOM_DVE_ROW_BASE + len(dve_ops.OPS)
    shas = {}
    for ver in ("v3", "v4"):
        s = DveOpSpec(name=name, opcode=opcode, uops=dve_lower(spec, ver=ver),
                      rd1_en=False)
        shas[ver] = s.sha(ver)
    op = dve_ops.DveOp(name, spec, subdim=False, uops_sha=shas)
    dve_ops.OPS.append(op)
    dve_ops._SUB_OPCODE_FOR_NAME[name] = opcode
    dve_ops.CUSTOM_DVE_SPECS[name] = spec
    dve_ops._EXP_POLY3_OP = op
    return op


EXP_POLY3 = _register_exp_poly()
P3_C0 = ALPHA ** 3 / 6.0
P3_C1 = ALPHA ** 2 / 2.0


def _build_program():
    nc = bacc.Bacc("TRN2", target_bir_lowering=False, debug=False,
                   num_devices=NCORES)

    din = {}

    def inp(name, shape, dt):
        din[name] = nc.dram_tensor(name, list(shape), dt,
                                   kind="ExternalInput").ap()
        return din[name]

    xq8_d = inp("xq8", (128, CP, 2, TQ), FP8)
    xqh_d = inp("xqh", (128, CT, TH), BF16)
    xkv8_d = inp("xkv8", (128, CP, 2, TKV), FP8)
    wq8_d = inp("wq8", (128, CP, 2, DL), FP8)
    wk8_d = inp("wk8", (128, CP, 2, DL), FP8)
    wv8_d = inp("wv8", (128, CP, 2, DL), FP8)
    wp8_d = inp("wp8", (128, 2, 2, DIM), FP8)
    wm18_d = inp("wm18", (128, CP, 2, MLPD), FP8)
    wm28_d = inp("wm28", (128, HP, 2, DIM), FP8)
    b1p8_d = inp("b1p8", (1, HT, 2, 128), FP8)
    sq_d = inp("sq", (DL,), F32)
    qb_d = inp("qb", (DL,), F32)
    pb_d = inp("pb", (DIM,), F32)
    b2_d = inp("b2", (DIM,), F32)
    out_d = nc.dram_tensor("out", [DIM, TH], F32, kind="ExternalOutput").ap()

    cc_in = [nc.dram_tensor(f"cc_in{i}", [2, DIM, TH // 2], FP8).ap()
             for i in range(2)]
    cc_rs = [nc.dram_tensor(f"cc_rs{i}", [DIM, TH // 2], FP8).ap()
             for i in range(2)]
    groups = [[0, 1], [2, 3], [4, 5], [6, 7]]

    with tile.TileContext(nc) as tc, ExitStack() as ctx:
        # ---- pools ----
        const = ctx.enter_context(tc.tile_pool(name="const", bufs=1))
        big = ctx.enter_context(tc.tile_pool(name="big", bufs=1))
        kvp = ctx.enter_context(tc.tile_pool(name="kvp", bufs=1))
        sexp_pool = ctx.enter_context(tc.tile_pool(name="sexp", bufs=9))
        med = ctx.enter_context(tc.tile_pool(name="med", bufs=1))
        sm = ctx.enter_context(tc.tile_pool(name="sm", bufs=2))
        sm2 = ctx.enter_context(tc.tile_pool(name="sm2", bufs=3))
        upool = ctx.enter_context(tc.tile_pool(name="upool", bufs=2))
        outp = ctx.enter_context(tc.tile_pool(name="outp", bufs=2))
        bc_pool = ctx.enter_context(tc.tile_pool(name="bc", bufs=2))

        # PSUM: ppS 2x2 banks (S pairs / MLP h), ppo 2x1 (PV accum),
        # ppK 2x1 (K/V/Q/proj/MLP2/stats)
        ppS = ctx.enter_context(tc.tile_pool(name="ppS", bufs=3, space="PSUM"))
        ppo = ctx.enter_context(tc.tile_pool(name="ppo", bufs=1, space="PSUM"))
        ppK = ctx.enter_context(tc.tile_pool(name="ppK", bufs=1, space="PSUM"))

        # ---- constants ----
        ones_bf = const.tile([128, 1], BF16)
        nc.vector.memset(ones_bf, 1.0)
        ones8_t = const.tile([128, 2, 16], FP8)
        nc.vector.memset(ones8_t, 1.0)
        ones8 = ones8_t[:, :, 0:1]
        ones8r = const.tile([1, 2, 256], FP8)
        nc.vector.memset(ones8r, 1.0)
        ident = const.tile([128, 128], BF16)
        make_identity(nc, ident)
        eps_t = const.tile([1, 1], F32)
        nc.vector.memset(eps_t, EPS)

        # ---- resident inputs / weights ----
        xq8 = big.tile([128, CP, 2, TQ], FP8, tag="xq8")
        nc.sync.dma_start(xq8, xq8_d)
        xkv8 = big.tile([128, CP, 2, TKV], FP8, tag="xkv8")
        nc.sync.dma_start(xkv8[:, :, :, 0:2048],
                          xkv8_d[:, :, :, 0:2048])
        nc.sync.dma_start(xkv8[:, :, :, 2048:TKV],
                          xkv8_d[:, :, :, 2048:TKV])
        wq8 = const.tile([128, CP, 2, DL], FP8)
        nc.sync.dma_start(wq8, wq8_d)
        wk8 = const.tile([128, CP, 2, DL], FP8)
        nc.sync.dma_start(wk8, wk8_d)
        wv8 = const.tile([128, CP, 2, DL], FP8)
        nc.sync.dma_start(wv8, wv8_d)
        xqh_sb = big.tile([128, CT, TH], BF16, tag="xqh")
        nc.sync.dma_start(xqh_sb, xqh_d)
        wp8 = const.tile([128, 2, 2, DIM], FP8)
        nc.sync.dma_start(wp8, wp8_d)
        wm18 = big.tile([128, CP, 2, MLPD], FP8, tag="wm1")
        nc.sync.dma_start(wm18, wm18_d)
        wm18r_d = nc.dram_tensor(
            "wm18r", [128, CP, 2, MLPD], FP8, kind="ExternalInput").ap()
        wm28r_d = nc.dram_tensor(
            "wm28r", [128, HP, 2, DIM], FP8, kind="ExternalInput").ap()
        b1p8 = const.tile([1, HT, 2, 128], FP8)
        nc.sync.dma_start(b1p8, b1p8_d)
        sq_sb = const.tile([128, DT], F32)
        nc.sync.dma_start(sq_sb, sq_d.rearrange("(a p) -> p a", p=128))
        qb_sb = const.tile([128, DT], F32)
        nc.sync.dma_start(qb_sb, qb_d.rearrange("(a p) -> p a", p=128))
        pb_sb = const.tile([128, CT], F32)
        nc.sync.dma_start(pb_sb, pb_d.rearrange("(a p) -> p a", p=128))
        b2_sb = const.tile([128, CT], F32)
        nc.sync.dma_start(b2_sb, b2_d.rearrange("(a p) -> p a", p=128))

        # ---- persistent attention tiles ----
        # K^T per d-group: [128(dl of 2 heads), TKV] fp8
        kt_sbs = [kvp.tile([128, TKV], FP8, tag=f"kt{d}", name=f"kt{d}")
                  for d in range(DT)]
        # V per d-group: [128(kt), group(16), slot(2), head(2), HD+1] fp8
        v_sbs = [kvp.tile([128, KTT // 2, 2, 2, HD + 1], FP8, tag=f"v{d}",
                          name=f"v{d}")
                 for d in range(DT)]
        # Q^T per d-group: [128(dl), slot(2), TQ] fp8, slot1 = zeros
        qt_sbs = [kvp.tile([128, 2, TQ], FP8, tag=f"qt{d}", name=f"qt{d}")
                  for d in range(DT)]
        for d in range(DT):
            nc.gpsimd.memset(qt_sbs[d][:, 1, :], 0.0)
            nc.gpsimd.memset(v_sbs[d][:, :, :, :, HD:HD + 1], 1.0)

        o_sb = med.tile([128, 8, DL], BF16, tag="osb")       # normalized O
        ot_sb = med.tile([128, DT, TQ], FP8, tag="ot")       # O^T for proj
        x1_sb = med.tile([128, CT, TQ], FP8, tag="x1")       # proj partial
        x1h = med.tile([128, CT, TH], BF16, tag="x1h")       # post-RS resid

        # ================= LN1 stats (from fp8 xq) ========================
        xsq8 = big.tile([128, CP, 2, TQ], FP8, tag="wm1r", name="xsq8")
        for p in range(CP):
            nc.gpsimd.tensor_tensor(xsq8[:, p], xq8[:, p], xq8[:, p],
                                    op=ALU.mult)

        mu_row = sm.tile([1, TQ], BF16, tag="st1")
        rs_row = sm.tile([1, TQ], BF16, tag="st1")
        for t2 in range(2):
            tsl = slice(t2 * 512, (t2 + 1) * 512)
            s_ps = ppK.tile([1, 512], F32, tag="k", name=f"sps{t2}")
            for p in range(CP):
                nc.tensor.matmul(s_ps, ones8, xq8[:, p, :, tsl],
                                 start=(p == 0), stop=(p == CP - 1),
                                 perf_mode=DRM)
            nc.vector.tensor_scalar_mul(mu_row[:, tsl], s_ps, 1.0 / DIM)
            q_ps = ppK.tile([1, 512], F32, tag="k", name=f"qps{t2}")
            for p in range(CP):
                nc.tensor.matmul(q_ps, ones8, xsq8[:, p, :, tsl],
                                 start=(p == 0), stop=(p == CP - 1),
                                 perf_mode=DRM)
            m2 = sm2.tile([1, 512], F32, tag="st2", name=f"m2{t2}")
            nc.vector.tensor_tensor(m2, mu_row[:, tsl], mu_row[:, tsl],
                                    op=ALU.mult)
            var = sm2.tile([1, 512], F32, tag="st2", name=f"var{t2}")
            nc.vector.scalar_tensor_tensor(var, q_ps, 1.0 / DIM, m2,
                                           op0=ALU.mult, op1=ALU.subtract)
            lnv = sm2.tile([1, 512], F32, tag="st2", name=f"lnv{t2}")
            nc.scalar.activation(lnv, var, AF.Ln, bias=eps_t[:1, :], scale=1.0)
            nc.scalar.activation(rs_row[:, tsl], lnv, AF.Exp, scale=-0.5)
        mu_b = bc_pool.tile([128, TQ], BF16, tag="bc")
        nc.gpsimd.partition_broadcast(mu_b, mu_row)
        rs_b = bc_pool.tile([128, TQ], BF16, tag="bc")
        nc.gpsimd.partition_broadcast(rs_b, rs_row)
        # wm18r reuses xsq8's slot (tag wm1r); DMA lands after stats read it
        wm18r = big.tile([128, CP, 2, MLPD], FP8, tag="wm1r", name="wm18r")
        nc.sync.dma_start(wm18r, wm18r_d)

        # ================= projections ====================================
        def make_q(d):
            dsl = slice(d * 128, (d + 1) * 128)
            qt = qt_sbs[d]
            for t2 in range(2):
                tsl = slice(t2 * 512, (t2 + 1) * 512)
                y_ps = ppK.tile([128, 512], F32, tag="k", name=f"y{d}{t2}")
                for p in range(CP):
                    nc.tensor.matmul(y_ps, wq8[:, p, :, dsl],
                                     xq8[:, p, :, tsl], start=(p == 0),
                                     stop=(p == CP - 1), perf_mode=DRM)
                u = upool.tile([128, 512], F32, tag="u", name=f"u{d}{t2}")
                nc.vector.scalar_tensor_tensor(u, mu_b[:, tsl],
                                               sq_sb[:, d:d + 1], y_ps,
                                               op0=ALU.mult, op1=ALU.subtract)
                v2 = upool.tile([128, 512], F32, tag="u", name=f"v{d}{t2}")
                nc.gpsimd.tensor_tensor(v2, u, rs_b[:, tsl], op=ALU.mult)
                nc.vector.tensor_scalar(qt[:, 0, tsl], v2, scalar1=-1.0,
                                        op0=ALU.mult,
                                        scalar2=qb_sb[:, d:d + 1],
                                        op1=ALU.add)

        def make_kv(d):
            dsl = slice(d * 128, (d + 1) * 128)
            kt, v_sb = kt_sbs[d], v_sbs[d]
            for ch in range(TKV // 512):
                ksl = slice(ch * 512, (ch + 1) * 512)
                k_ps = ppK.tile([128, 512], F32, tag="k", name=f"k{d}{ch}")
                for p in range(CP):
                    nc.tensor.matmul(k_ps, wk8[:, p, :, dsl],
                                     xkv8[:, p, :, ksl], start=(p == 0),
                                     stop=(p == CP - 1), perf_mode=DRM)
                nc.scalar.copy(kt[:, ksl], k_ps)
                v_ps = ppK.tile([128, 4, 128], F32, tag="k", name=f"vp{d}{ch}")
                for j in range(4):
                    ktt = ch * 4 + j
                    ktsl = slice(ktt * 128, (ktt + 1) * 128)
                    for p in range(CP):
                        nc.tensor.matmul(v_ps[:, j, :],
                                         xkv8[:, p, :, ktsl],
                                         wv8[:, p, :, dsl], start=(p == 0),
                                         stop=(p == CP - 1), perf_mode=DRM)
                # [128,4,128] -> v_sb[:, 2ch:2ch+2, :, :, 0:HD]
                nc.vector.tensor_copy(
                    v_sb[:, 2 * ch:2 * ch + 2, :, :, 0:HD],
                    v_ps.rearrange("q (g s) (h x) -> q g s h x", g=2, h=2))

        # ================= attention ======================================
        def attn_one(d, hh, t2):
            qt, kt, v_sb = qt_sbs[d], kt_sbs[d], v_sbs[d]
            tsl = slice(t2 * 512, (t2 + 1) * 512)
            rsl = slice(hh * 64, hh * 64 + 64)
            o_ps = ppo.tile([128, 4, HD + 1], F32, tag="o",
                            name=f"ops{d}{hh}{t2}")
            q_dr = qt[rsl, :, tsl]
            NW = 8                      # groups per wave
            for w in range(KTT // 2 // NW):
                sexps = []
                for gg in range(NW):
                    g = w * NW + gg
                    s_ps = ppS.tile([128, 2, 512], F32, tag="s",
                                    name=f"s{d}{hh}{t2}{g}")
                    for i in range(2):
                        ktt = 2 * g + i
                        kbase = kt[rsl, ktt * 128:(ktt + 1) * 128]
                        k_dr = bass.AP(tensor=kbase.tensor,
                                       offset=kbase.offset,
                                       ap=[kbase.ap[0], [0, 2], kbase.ap[1]])
                        nc.tensor.matmul(s_ps[:, i, :], k_dr, q_dr,
                                         start=True, stop=True, perf_mode=DRM)
                    sexp = sexp_pool.tile([128, 2, 512], FP8, tag="se",
                                          name=f"se{d}{hh}{t2}{g}")
                    sexps.append(sexp)
                    if g % 2 == 0:
                        nc.scalar.activation(sexp, s_ps, AF.Exp, scale=ALPHA)
                    else:
                        nc.vector._custom_dve(EXP_POLY3, out=sexp, in0=s_ps,
                                              s0=P3_C0, s1=P3_C1, imm2=ALPHA)
                for gg in range(NW):
                    g = w * NW + gg
                    for tt in range(4):
                        nc.tensor.matmul(
                            o_ps[:, tt, :],
                            sexps[gg][:, :, tt * 128:(tt + 1) * 128],
                            v_sb[:, g, :, hh, :],
                            start=(g == 0), stop=(g == KTT // 2 - 1),
                            perf_mode=DRM)
            # normalize: o = o_raw * (1/denom), stride-0 reciprocal view
            rr = sm.tile([128, 4], F32, tag="rr", name=f"rr{d}{hh}{t2}")
            nc.vector.reciprocal(rr, o_ps[:, :, HD])
            rrv = bass.AP(tensor=rr.tensor, offset=rr.offset,
                          ap=[rr.ap[0], [1, 4], [0, HD]])
            h = 2 * d + hh
            nc.vector.tensor_tensor(
                o_sb[:, t2 * 4:(t2 + 1) * 4, h * 64:(h + 1) * 64],
                o_ps[:, :, 0:HD], rrv, op=ALU.mult)

        # ================= proj + ReduceScatter ===========================
        rsx_tiles = []

        def proj_rs(t2):
            tsl = slice(t2 * 512, (t2 + 1) * 512)
            for tt in range(4):
                for d in range(DT):
                    t_ps = ppK.tile([128, 128], BF16, tag="k",
                                    name=f"tp{t2}{tt}{d}")
                    nc.tensor.transpose(
                        t_ps, o_sb[:, t2 * 4 + tt, d * 128:(d + 1) * 128],
                        ident)
                    nc.vector.tensor_copy(
                        ot_sb[:, d,
                              t2 * 512 + tt * 128:t2 * 512 + (tt + 1) * 128],
                        t_ps)
            for c in range(CT):
                csl = slice(c * 128, (c + 1) * 128)
                p_ps = ppK.tile([128, 512], F32, tag="k", name=f"pj{c}{t2}")
                for p in range(2):
                    if p == 0:
                        rhs = ot_sb[:, 0:2, tsl]
                    else:
                        rb = ot_sb[:, 2, tsl]
                        rhs = bass.AP(tensor=rb.tensor, offset=rb.offset,
                                      ap=[rb.ap[0], [0, 2], rb.ap[1]])
                    nc.tensor.matmul(p_ps, wp8[:, p, :, csl], rhs,
                                     start=(p == 0), stop=(p == 1),
                                     perf_mode=DRM)
                nc.vector.tensor_copy(x1_sb[:, c, tsl], p_ps)
            for blk in range(2):
                nc.gpsimd.dma_start(
                    cc_in[t2][blk].rearrange("(a p) t -> p a t", p=128),
                    x1_sb[:, :, t2 * 512 + blk * 256:t2 * 512 + (blk + 1) * 256])
            nc.gpsimd.collective_compute(
                "ReduceScatter", ALU.add, ins=[cc_in[t2]], outs=[cc_rs[t2]],
                replica_groups=groups)
            rsx = upool.tile([128, CT, 256], FP8, tag="rsx", name=f"rsx{t2}")
            nc.gpsimd.dma_start(rsx, cc_rs[t2].rearrange("(a p) t -> p a t",
                                                         p=128))
            rsx_tiles.append(rsx)

        # ================= schedule: attention phase ======================
        make_q(0)
        make_kv(0)
        attn_one(0, 0, 0)
        attn_one(0, 0, 1)
        make_q(1)
        make_kv(1)
        attn_one(0, 1, 0)
        attn_one(0, 1, 1)
        make_q(2)
        make_kv(2)
        # xkv8 is dead now; its slot hosts wm28 for the MLP tail
        wm28 = big.tile([128, HP, 2, DIM], FP8, tag="xkv8", name="wm28")
        nc.sync.dma_start(wm28, wm28_d)
        attn_one(1, 0, 0)
        attn_one(1, 1, 0)
        attn_one(2, 0, 0)
        attn_one(2, 1, 0)
        proj_rs(0)

        # ================= post-RS: residual + LN2 + MLP ==================
        h8s = []

        def mlp_q(q):
            qsl = slice(q * 256, (q + 1) * 256)
            rsx = rsx_tiles[q]
            for c in range(CT):
                nc.vector.affine_then_add(x1h[:, c, qsl], rsx[:, c, :],
                                          xqh_sb[:, c, qsl],
                                          scale=INV_SW2,
                                          bias=pb_sb[:, c:c + 1])
            # LN2 stats (bf16 ones-matmuls)
            x2h = upool.tile([128, CT, 256], FP8, tag="x2h", name=f"x2h{q}")
            for c in range(CT):
                nc.gpsimd.tensor_tensor(x2h[:, c, :], x1h[:, c, qsl],
                                        x1h[:, c, qsl], op=ALU.mult)
            s2_ps = ppK.tile([1, 256], F32, tag="k", name=f"s2{q}")
            for c in range(CT):
                nc.tensor.matmul(s2_ps, ones_bf, x1h[:, c, qsl],
                                 start=(c == 0), stop=(c == CT - 1))
            mu2 = sm2.tile([1, 256], F32, tag="st2", name=f"mu2{q}")
            nc.vector.tensor_scalar_mul(mu2, s2_ps, 1.0 / DIM)
            mu2bf = sm2.tile([1, 256], BF16, tag="st2", name=f"mu2bf{q}")
            nc.vector.tensor_copy(mu2bf, mu2)
            mu2_b = bc_pool.tile([128, 256], BF16, tag="bc2", name=f"mu2b{q}")
            nc.gpsimd.partition_broadcast(mu2_b, mu2bf)
            q2_ps = ppK.tile([1, 256], F32, tag="k", name=f"q2{q}")
            for c in range(CT):
                nc.tensor.matmul(q2_ps, ones_bf, x2h[:, c, :],
                                 start=(c == 0), stop=(c == CT - 1))
            m22 = sm2.tile([1, 256], F32, tag="st2", name=f"m22{q}")
            nc.vector.tensor_tensor(m22, mu2, mu2, op=ALU.mult)
            var2 = sm2.tile([1, 256], F32, tag="st2", name=f"var2{q}")
            nc.vector.scalar_tensor_tensor(var2, q2_ps, 1.0 / DIM, m22,
                                           op0=ALU.mult, op1=ALU.subtract)
            lnv2 = sm2.tile([1, 256], F32, tag="st2", name=f"lnv2{q}")
            nc.scalar.activation(lnv2, var2, AF.Ln, bias=eps_t[:1, :],
                                 scale=1.0)
            rs2 = sm2.tile([1, 256], F32, tag="st2", name=f"rs2{q}")
            nc.scalar.activation(rs2, lnv2, AF.Exp, scale=-0.5)
            rs2bf = sm2.tile([1, 256], BF16, tag="st2", name=f"rs2bf{q}")
            nc.vector.tensor_copy(rs2bf, rs2)
            rs2_b = bc_pool.tile([128, 256], BF16, tag="bc2", name=f"rs2b{q}")
            nc.gpsimd.partition_broadcast(rs2_b, rs2bf)

            def bview(t):
                a = t[:, :]
                return bass.AP(tensor=a.tensor, offset=a.offset,
                               ap=[a.ap[0], [0, CT], a.ap[1]])

            tx = upool.tile([128, CT, 256], BF16, tag="u", name=f"tx{q}")
            nc.vector.tensor_tensor(tx, x1h[:, :, qsl], bview(mu2_b),
                                    op=ALU.subtract)
            xn2b = upool.tile([128, CT, 256], BF16, tag="u", name=f"xn2b{q}")
            nc.vector.tensor_tensor(xn2b, tx, bview(rs2_b), op=ALU.mult)
            xn2 = upool.tile([128, CP, 2, 256], FP8, tag="xn2", name=f"xn2{q}")
            nc.gpsimd.tensor_copy(xn2.rearrange("r p s t -> r (p s) t"), xn2b)
            xn2r = upool.tile([128, CP, 2, 256], FP8, tag="xn2r",
                              name=f"xn2r{q}")
            nc.gpsimd.tensor_tensor(xn2r.rearrange("r p s t -> r (p s) t"),
                                    xn2b,
                                    xn2.rearrange("r p s t -> r (p s) t"),
                                    op=ALU.subtract)

            # reuse dead slots: q0 -> xq8 (dead after make_q/LN1),
            # q1 -> o_sb (dead after proj(1) transposes)
            if q == 0:
                h8 = big.tile([128, HP, 2, 256], FP8, tag="xq8", name="h8q0")
            else:
                h8 = med.tile([128, HP, 2, 256], FP8, tag="osb", name="h8q1")
            h8s.append(h8)
            for mg in range(HP // 2):
                h_ps = ppS.tile([128, 4, 256], F32, tag="s", name=f"h{q}{mg}")
                for mi in range(4):
                    m = 4 * mg + mi
                    msl = slice(m * 128, (m + 1) * 128)
                    nc.tensor.matmul(h_ps[:, mi, :], b1p8[:, m, :, :], ones8r,
                                     start=True, stop=False, perf_mode=DRM)
                    for p in range(CP):
                        nc.tensor.matmul(h_ps[:, mi, :], wm18[:, p, :, msl],
                                         xn2[:, p, :, :], start=False,
                                         stop=False, perf_mode=DRM)
                    for p in range(CP):
                        nc.tensor.matmul(h_ps[:, mi, :], wm18r[:, p, :, msl],
                                         xn2[:, p, :, :], start=False,
                                         stop=False, perf_mode=DRM)
                    for p in range(CP):
                        nc.tensor.matmul(h_ps[:, mi, :], wm18[:, p, :, msl],
                                         xn2r[:, p, :, :], start=False,
                                         stop=(p == CP - 1), perf_mode=DRM)
                nc.scalar.activation(
                    h8[:, 2 * mg:2 * mg + 2, :, :],
                    h_ps.rearrange("r (a s) t -> r a s t", a=2), AF.Gelu,
                    scale=INV_SW)

        def mlp2_q(q):
            qsl = slice(q * 256, (q + 1) * 256)
            h8 = h8s[q]
            for cp in range(CP):
                o2_ps = ppK.tile([128, 2, 256], F32, tag="k",
                                 name=f"o2{q}{cp}")
                for ci in range(2):
                    c = 2 * cp + ci
                    csl = slice(c * 128, (c + 1) * 128)
                    wm2rc = upool.tile([128, HP, 2, 128], FP8, tag="wm2rc",
                                       name=f"wm2rc{q}{cp}{ci}")
                    nc.sync.dma_start(wm2rc, wm28r_d[:, :, :, csl])
                    for hp in range(HP):
                        nc.tensor.matmul(o2_ps[:, ci, :],
                                         wm28[:, hp, :, csl],
                                         h8[:, hp, :, :], start=(hp == 0),
                                         stop=False, perf_mode=DRM)
                    for hp in range(HP):
                        nc.tensor.matmul(o2_ps[:, ci, :],
                                         wm2rc[:, hp, :, :],
                                         h8[:, hp, :, :], start=False,
                                         stop=(hp == HP - 1), perf_mode=DRM)
                for ci in range(2):
                    c = 2 * cp + ci
                    csl = slice(c * 128, (c + 1) * 128)
                    fin = outp.tile([128, 256], F32, tag="outp",
                                    name=f"fin{q}{cp}{ci}")
                    nc.vector.affine_then_add(fin, o2_ps[:, ci, :],
                                              x1h[:, c, qsl], scale=INV_SW,
                                              bias=b2_sb[:, c:c + 1])
                    nc.sync.dma_start(out_d[csl, qsl], fin)

        attn_one(1, 0, 1)
        attn_one(1, 1, 1)
        attn_one(2, 0, 1)
        attn_one(2, 1, 1)
        proj_rs(1)
        mlp_q(0)
        mlp2_q(0)
        mlp_q(1)
        mlp2_q(1)

    nc.compile()
    return nc


def _prep_inputs(inputs):
    """Build the 8 per-core in_maps from the full-size inputs."""
    f8 = np.float64
    xq = np.asarray(inputs["xq"], np.float32)
    xkv = np.asarray(inputs["xkv"], np.float32)
    n1w = np.asarray(inputs["norm1_w"], f8); n1b = np.asarray(inputs["norm1_b"], f8)
    kv_w = np.asarray(inputs["kv_w"], f8); kv_b = np.asarray(inputs["kv_b"], f8)
    q_w = np.asarray(inputs["q_w"], f8); q_b = np.asarray(inputs["q_b"], f8)
    p_w = np.asarray(inputs["proj_w"], f8); p_b = np.asarray(inputs["proj_b"], f8)
    n2w = np.asarray(inputs["norm2_w"], f8); n2b = np.asarray(inputs["norm2_b"], f8)
    w1 = np.asarray(inputs["mlp_w1"], f8); b1 = np.asarray(inputs["mlp_b1"], f8)
    w2 = np.asarray(inputs["mlp_w2"], f8); b2 = np.asarray(inputs["mlp_b2"], f8)

    def cpair(mat, ncols):
        """[768, ncols] -> [128, 3, 2, ncols] channel-pair layout."""
        return np.ascontiguousarray(
            mat.reshape(CP, 2, 128, ncols).transpose(2, 0, 1, 3))

    wq_f = n1w[:, None] * q_w                                # [768, 768]
    qb_f = (q_b + n1b @ q_w) * SW                            # [768]
    kvw = kv_w.reshape(DIM, 2, NH, HD)
    kvb = kv_b.reshape(2, NH, HD)
    # v-bias folded into proj bias: sum over ALL heads
    pb_f = p_b + kvb[1].reshape(NH * HD) @ p_w               # [768]
    wm1_f = n2w[:, None] * w1
    b1_f = b1 + n2b @ w1
    wm18 = cpair((SW * wm1_f), MLPD).astype(F8)
    wm18r = cpair(
        SW * wm1_f - cpair(SW * wm1_f, MLPD).astype(F8).astype(np.float64)
        .transpose(1, 2, 0, 3).reshape(DIM, MLPD), MLPD).astype(F8)
    wm28 = np.ascontiguousarray(
        (SW * w2).reshape(HP, 2, 128, DIM).transpose(2, 0, 1, 3)).astype(F8)
    wm28r = np.ascontiguousarray(
        ((SW * w2) - (SW * w2).astype(F8).astype(np.float64))
        .reshape(HP, 2, 128, DIM).transpose(2, 0, 1, 3)).astype(F8)
    b1p8 = np.zeros((1, HT, 2, 128), F8)
    b1p8[0, :, 0, :] = (SW * b1_f).reshape(HT, 128).astype(F8)

    maps = []
    for core in range(NCORES):
        b, half = divmod(core, 2)
        hs = slice(half * NHL, (half + 1) * NHL)
        tidx = np.r_[half * 256:(half + 1) * 256,
                     512 + half * 256:512 + (half + 1) * 256]
        xqT = np.ascontiguousarray(xq[b].reshape(TQ, DIM).T)
        xkvT = np.ascontiguousarray(xkv[b].reshape(TKV, DIM).T)
        wq_c = cpair(SW * np.ascontiguousarray(
            wq_f.reshape(DIM, NH, HD)[:, hs].reshape(DIM, DL)), DL).astype(F8)
        wp_loc = SW * p_w.reshape(NH, HD, DIM)[hs].reshape(DL, DIM)
        wp_pad = np.zeros((4, 128, DIM), f8)
        wp_pad[0:DT] = wp_loc.reshape(DT, 128, DIM)
        wp8 = np.ascontiguousarray(
            wp_pad.reshape(2, 2, 128, DIM).transpose(2, 0, 1, 3)).astype(F8)
        m = {
            "xq8": cpair(xqT, TQ).astype(F8),
            "xqh": np.ascontiguousarray(
                xqT[:, tidx].reshape(CT, 128, TH).transpose(1, 0, 2)
            ).astype(BF),
            "xkv8": cpair(xkvT, TKV).astype(F8),
            "wq8": wq_c,
            "wk8": cpair(SW * np.ascontiguousarray(
                kvw[:, 0, hs].reshape(DIM, DL)), DL).astype(F8),
            "wv8": cpair(SW * np.ascontiguousarray(
                kvw[:, 1, hs].reshape(DIM, DL)), DL).astype(F8),
            "wp8": wp8,
            "wm18": wm18,
            "wm18r": wm18r,
            "wm28": wm28,
            "wm28r": wm28r,
            "b1p8": b1p8,
            # sq: column sums of the fp8 wq (exact, f64), [DL]
            "sq": wq_c.astype(np.float64).sum(axis=(0, 1, 2)).astype(
                np.float32),
            "qb": np.ascontiguousarray(
                qb_f.reshape(NH, HD)[hs].reshape(DL)).astype(np.float32),
            "pb": pb_f.astype(np.float32),
            "b2": b2.astype(np.float32),
        }
        maps.append(m)
    return maps


def kernel(**inputs):
    if "nc" not in _CACHE:
        _CACHE["nc"] = _build_program()
    nc = _CACHE["nc"]
    maps = _prep_inputs(inputs)
    res = run_bass_kernel_spmd(nc, maps, core_ids=list(range(NCORES)))
    out = np.zeros((B, TQ, DIM), np.float32)
    for core in range(NCORES):
        b, half = divmod(core, 2)
        tidx = np.r_[half * 256:(half + 1) * 256,
                     512 + half * 256:512 + (half + 1) * 256]
        x2T = res.results[core]["out"]          # [768, 512]
        out[b, tidx, :] = x2T.T
    return out.reshape(B, 32, 32, DIM)


# revision 25
# speedup vs baseline: 1.1559x; 1.0042x over previous
"""CrossBlock (cross-attention transformer block) on 8 TRN2 NeuronCores.

Sharding: 4 batch elements x 2 cores each (tensor-parallel over heads).
Core c = 2*b + half handles batch b; half selects heads 6*half..6*half+5.

v2: fp8(e4m3) DoubleRow matmuls everywhere (2 k-tiles per instruction),
softmax exp split between the Scalar (ACT) engine (true exp) and the Vector
(DVE) engine (one-instruction cubic-poly exp via a custom DVE op), psum->sbuf
copies spread over ACT/DVE, sbuf-only elementwise work on GPSIMD (Pool).
LayerNorm rstd via exp(-0.5*ln(var+eps)) so the attention phase stays within
one ACT function table. Weights are host-scaled by SW=32 to keep fp8 operands
out of the subnormal range; the inverse scales fold into activation-scale /
affine ops. Biases that only shift logits uniformly per query (k bias) are
dropped; v bias is folded into the proj bias on the host.

Per-core flow:
  LN1 stats (fp8 ones-matmuls) -> Q/K/V projections (fp8 DR) -> attention
  (S^T fp8 DR with a zero k-slot; exp ACT/DVE split; P@V fp8 DR with a ones
  column for the softmax denominator; normalize via tensor-tensor divide with
  a stride-0 denominator view) -> proj partial -> pairwise fp8 ReduceScatter
  (token split) -> residual + LN2 + MLP (fp8 DR, gelu on ACT, bias via a
  K=1 matmul) -> f32 output.
"""

import numpy as np
import ml_dtypes
from contextlib import ExitStack

import concourse.bass as bass
import concourse.tile as tile
from concourse import bacc, mybir
from concourse.bass_utils import run_bass_kernel_spmd
from concourse.masks import make_identity

F32 = mybir.dt.float32
BF16 = mybir.dt.bfloat16
FP8 = mybir.dt.float8e4
AF = mybir.ActivationFunctionType
ALU = mybir.AluOpType
DRM = mybir.MatmulPerfMode.DoubleRow
BF = ml_dtypes.bfloat16
F8 = ml_dtypes.float8_e4m3

DIM = 768
NH = 12
HD = 64
MLPD = 3072
EPS = 1e-5
B = 4
TQ = 1024          # query tokens per batch element
TKV = 4096         # kv tokens per batch element
NHL = NH // 2      # heads per core (6)
DL = NHL * HD      # local head cols (384)
TH = TQ // 2       # token half for the MLP stage (512)
CT = DIM // 128    # channel tiles (6)
CP = CT // 2       # channel pairs (3)
DT = DL // 128     # local head-pair groups (3)
HT = MLPD // 128   # hidden tiles (24)
HP = HT // 2       # hidden pairs (12)
KTT = TKV // 128   # kv token tiles (32)
NCORES = 8

SW = 32.0                      # host-side fp8 weight scale
ASC = HD ** -0.5               # attention scale (1/8)
ALPHA = ASC / (SW * SW)        # fold of attn scale + q/k weight scales
INV_SW = 1.0 / SW
INV_SW2 = 1.0 / (SW * SW)

# exp split: one ACT (true exp) per EXP_RATIO kt-groups, rest DVE poly.
EXP_ACT_OF = 8     # of every 16 groups, this many go to ACT

_CACHE = {}

# ---------------------------------------------------------------------------
# custom DVE op: one-instruction cubic exp approximation
#   f(s) = C0*s^3 + C1*s^2 + imm2*s + 1  (= Taylor of exp(imm2*s) when
#   C0=imm2^3/6, C1=imm2^2/2). The constant term 1 is exact, which keeps
#   softmax normalization consistent with the ACT-exp share.
# ---------------------------------------------------------------------------
import concourse.dve_ops as dve_ops
from concourse.dve_spec import Spec, Src0, C0, C1, C2, One, lower as dve_lower
from concourse.dve_uop import DveOpSpec


def _register_exp_poly():
    if hasattr(dve_ops, "_EXP_POLY3_OP"):
        return dve_ops._EXP_POLY3_OP
    body = ((Src0 * C0 + C1) * Src0 + C2) * Src0 + One
    spec = Spec(
        body=body,
        reference=lambda in0, in1, c0, c1, c2: (
            ((in0.astype(np.float32) * c0 + c1) * in0 + c2) * in0 + 1.0
        ),
    )
    name = "EXP_POLY3"
    opcode = dve_ops._CUSTOM_DVE_ROW_BASE + len(dve_ops.OPS)
    shas = {}
    for ver in ("v3", "v4"):
        s = DveOpSpec(name=name, opcode=opcode, uops=dve_lower(spec, ver=ver),
                      rd1_en=False)
        shas[ver] = s.sha(ver)
    op = dve_ops.DveOp(name, spec, subdim=False, uops_sha=shas)
    dve_ops.OPS.append(op)
    dve_ops._SUB_OPCODE_FOR_NAME[name] = opcode
    dve_ops.CUSTOM_DVE_SPECS[name] = spec
    dve_ops._EXP_POLY3_OP = op
    return op


EXP_POLY3 = _register_exp_poly()
P3_C0 = ALPHA ** 3 / 6.0
P3_C1 = ALPHA ** 2 / 2.0


def _build_program():
    nc = bacc.Bacc("TRN2", target_bir_lowering=False, debug=False,
                   num_devices=NCORES)

    din = {}

    def inp(name, shape, dt):
        din[name] = nc.dram_tensor(name, list(shape), dt,
                                   kind="ExternalInput").ap()
        return din[name]

    xq8_d = inp("xq8", (128, CP, 2, TQ), FP8)
    xqh_d = inp("xqh", (128, CT, TH), BF16)
    xkv8_d = inp("xkv8", (128, CP, 2, TKV), FP8)
    wq8_d = inp("wq8", (128, CP, 2, DL), FP8)
    wk8_d = inp("wk8", (128, CP, 2, DL), FP8)
    wv8_d = inp("wv8", (128, CP, 2, DL), FP8)
    wp8_d = inp("wp8", (128, 2, 2, DIM), FP8)
    wm18_d = inp("wm18", (128, CP, 2, MLPD), FP8)
    wm28_d = inp("wm28", (128, HP, 2, DIM), FP8)
    b1p8_d = inp("b1p8", (1, HT, 2, 128), FP8)
    sq_d = inp("sq", (DL,), F32)
    qb_d = inp("qb", (DL,), F32)
    pb_d = inp("pb", (DIM,), F32)
    b2_d = inp("b2", (DIM,), F32)
    out_d = nc.dram_tensor("out", [DIM, TH], F32, kind="ExternalOutput").ap()

    cc_in = [nc.dram_tensor(f"cc_in{i}", [2, DIM, TH // 2], FP8).ap()
             for i in range(2)]
    cc_rs = [nc.dram_tensor(f"cc_rs{i}", [DIM, TH // 2], FP8).ap()
             for i in range(2)]
    groups = [[0, 1], [2, 3], [4, 5], [6, 7]]

    with tile.TileContext(nc) as tc, ExitStack() as ctx:
        # ---- pools ----
        const = ctx.enter_context(tc.tile_pool(name="const", bufs=1))
        big = ctx.enter_context(tc.tile_pool(name="big", bufs=1))
        kvp = ctx.enter_context(tc.tile_pool(name="kvp", bufs=1))
        sexp_pool = ctx.enter_context(tc.tile_pool(name="sexp", bufs=9))
        med = ctx.enter_context(tc.tile_pool(name="med", bufs=1))
        sm = ctx.enter_context(tc.tile_pool(name="sm", bufs=2))
        sm2 = ctx.enter_context(tc.tile_pool(name="sm2", bufs=3))
        upool = ctx.enter_context(tc.tile_pool(name="upool", bufs=2))
        outp = ctx.enter_context(tc.tile_pool(name="outp", bufs=2))
        bc_pool = ctx.enter_context(tc.tile_pool(name="bc", bufs=2))

        # PSUM: ppS 2x2 banks (S pairs / MLP h), ppo 2x1 (PV accum),
        # ppK 2x1 (K/V/Q/proj/MLP2/stats)
        ppS = ctx.enter_context(tc.tile_pool(name="ppS", bufs=3, space="PSUM"))
        ppo = ctx.enter_context(tc.tile_pool(name="ppo", bufs=1, space="PSUM"))
        ppK = ctx.enter_context(tc.tile_pool(name="ppK", bufs=1, space="PSUM"))

        # ---- constants ----
        ones_bf = const.tile([128, 1], BF16)
        nc.vector.memset(ones_bf, 1.0)
        ones8_t = const.tile([128, 2, 16], FP8)
        nc.vector.memset(ones8_t, 1.0)
        ones8 = ones8_t[:, :, 0:1]
        ones8r = const.tile([1, 2, 256], FP8)
        nc.vector.memset(ones8r, 1.0)
        ident = const.tile([128, 128], BF16)
        make_identity(nc, ident)
        eps_t = const.tile([1, 1], F32)
        nc.vector.memset(eps_t, EPS)

        # ---- resident inputs / weights ----
        xq8 = big.tile([128, CP, 2, TQ], FP8, tag="xq8")
        nc.sync.dma_start(xq8, xq8_d)
        xkv8 = big.tile([128, CP, 2, TKV], FP8, tag="xkv8")
        nc.sync.dma_start(xkv8[:, :, :, 0:2048],
                          xkv8_d[:, :, :, 0:2048])
        nc.sync.dma_start(xkv8[:, :, :, 2048:TKV],
                          xkv8_d[:, :, :, 2048:TKV])
        wq8 = const.tile([128, CP, 2, DL], FP8)
        nc.sync.dma_start(wq8, wq8_d)
        wk8 = const.tile([128, CP, 2, DL], FP8)
        nc.sync.dma_start(wk8, wk8_d)
        wv8 = const.tile([128, CP, 2, DL], FP8)
        nc.sync.dma_start(wv8, wv8_d)
        xqh_sb = big.tile([128, CT, TH], BF16, tag="xqh")
        nc.sync.dma_start(xqh_sb, xqh_d)
        wp8 = const.tile([128, 2, 2, DIM], FP8)
        nc.sync.dma_start(wp8, wp8_d)
        wm18 = big.tile([128, CP, 2, MLPD], FP8, tag="wm1")
        nc.sync.dma_start(wm18, wm18_d)
        wm18r_d = nc.dram_tensor(
            "wm18r", [128, CP, 2, MLPD], FP8, kind="ExternalInput").ap()
        wm28r_d = nc.dram_tensor(
            "wm28r", [128, HP, 2, DIM], FP8, kind="ExternalInput").ap()
        b1p8 = const.tile([1, HT, 2, 128], FP8)
        nc.sync.dma_start(b1p8, b1p8_d)
        sq_sb = const.tile([128, DT], F32)
        nc.sync.dma_start(sq_sb, sq_d.rearrange("(a p) -> p a", p=128))
        qb_sb = const.tile([128, DT], F32)
        nc.sync.dma_start(qb_sb, qb_d.rearrange("(a p) -> p a", p=128))
        pb_sb = const.tile([128, CT], F32)
        nc.sync.dma_start(pb_sb, pb_d.rearrange("(a p) -> p a", p=128))
        b2_sb = const.tile([128, CT], F32)
        nc.sync.dma_start(b2_sb, b2_d.rearrange("(a p) -> p a", p=128))

        # ---- persistent attention tiles ----
        # K^T per d-group: [128(dl of 2 heads), TKV] fp8
        kt_sbs = [kvp.tile([128, TKV], FP8, tag=f"kt{d}", name=f"kt{d}")
                  for d in range(DT)]
        # V per d-group: [128(kt), group(16), slot(2), head(2), HD+1] fp8
        v_sbs = [kvp.tile([128, KTT // 2, 2, 2, HD + 1], FP8, tag=f"v{d}",
                          name=f"v{d}")
                 for d in range(DT)]
        # Q^T per d-group: [128(dl), slot(2), TQ] fp8, slot1 = zeros
        qt_sbs = [kvp.tile([128, 2, TQ], FP8, tag=f"qt{d}", name=f"qt{d}")
                  for d in range(DT)]
        for d in range(DT):
            nc.gpsimd.memset(qt_sbs[d][:, 1, :], 0.0)
            nc.gpsimd.memset(v_sbs[d][:, :, :, :, HD:HD + 1], 1.0)

        o_sb = med.tile([128, 8, DL], BF16, tag="osb")       # normalized O
        ot_sb = med.tile([128, DT, TQ], FP8, tag="ot")       # O^T for proj
        x1_sb = med.tile([128, CT, TQ], FP8, tag="x1")       # proj partial
        x1h = med.tile([128, CT, TH], BF16, tag="x1h")       # post-RS resid

        # ================= LN1 stats (from fp8 xq) ========================
        xsq8 = big.tile([128, CP, 2, TQ], FP8, tag="wm1r", name="xsq8")
        for p in range(CP):
            nc.gpsimd.tensor_tensor(xsq8[:, p], xq8[:, p], xq8[:, p],
                                    op=ALU.mult)

        mu_row = sm.tile([1, TQ], BF16, tag="st1")
        rs_row = sm.tile([1, TQ], BF16, tag="st1")
        for t2 in range(2):
            tsl = slice(t2 * 512, (t2 + 1) * 512)
            s_ps = ppK.tile([1, 512], F32, tag="k", name=f"sps{t2}")
            for p in range(CP):
                nc.tensor.matmul(s_ps, ones8, xq8[:, p, :, tsl],
                                 start=(p == 0), stop=(p == CP - 1),
                                 perf_mode=DRM)
            nc.vector.tensor_scalar_mul(mu_row[:, tsl], s_ps, 1.0 / DIM)
            q_ps = ppK.tile([1, 512], F32, tag="k", name=f"qps{t2}")
            for p in range(CP):
                nc.tensor.matmul(q_ps, ones8, xsq8[:, p, :, tsl],
                                 start=(p == 0), stop=(p == CP - 1),
                                 perf_mode=DRM)
            m2 = sm2.tile([1, 512], F32, tag="st2", name=f"m2{t2}")
            nc.vector.tensor_tensor(m2, mu_row[:, tsl], mu_row[:, tsl],
                                    op=ALU.mult)
            var = sm2.tile([1, 512], F32, tag="st2", name=f"var{t2}")
            nc.vector.scalar_tensor_tensor(var, q_ps, 1.0 / DIM, m2,
                                           op0=ALU.mult, op1=ALU.subtract)
            lnv = sm2.tile([1, 512], F32, tag="st2", name=f"lnv{t2}")
            nc.scalar.activation(lnv, var, AF.Ln, bias=eps_t[:1, :], scale=1.0)
            nc.scalar.activation(rs_row[:, tsl], lnv, AF.Exp, scale=-0.5)
        mu_b = bc_pool.tile([128, TQ], BF16, tag="bc")
        nc.gpsimd.partition_broadcast(mu_b, mu_row)
        rs_b = bc_pool.tile([128, TQ], BF16, tag="bc")
        nc.gpsimd.partition_broadcast(rs_b, rs_row)
        # wm18r reuses xsq8's slot (tag wm1r); DMA lands after stats read it
        wm18r = big.tile([128, CP, 2, MLPD], FP8, tag="wm1r", name="wm18r")
        nc.sync.dma_start(wm18r, wm18r_d)

        # ================= projections ====================================
        def make_q(d):
            dsl = slice(d * 128, (d + 1) * 128)
            qt = qt_sbs[d]
            for t2 in range(2):
                tsl = slice(t2 * 512, (t2 + 1) * 512)
                y_ps = ppK.tile([128, 512], F32, tag="k", name=f"y{d}{t2}")
                for p in range(CP):
                    nc.tensor.matmul(y_ps, wq8[:, p, :, dsl],
                                     xq8[:, p, :, tsl], start=(p == 0),
                                     stop=(p == CP - 1), perf_mode=DRM)
                u = upool.tile([128, 512], F32, tag="u", name=f"u{d}{t2}")
                nc.vector.scalar_tensor_tensor(u, mu_b[:, tsl],
                                               sq_sb[:, d:d + 1], y_ps,
                                               op0=ALU.mult, op1=ALU.subtract)
                v2 = upool.tile([128, 512], F32, tag="u", name=f"v{d}{t2}")
                nc.gpsimd.tensor_tensor(v2, u, rs_b[:, tsl], op=ALU.mult)
                nc.vector.tensor_scalar(qt[:, 0, tsl], v2, scalar1=-1.0,
                                        op0=ALU.mult,
                                        scalar2=qb_sb[:, d:d + 1],
                                        op1=ALU.add)

        def make_kv(d):
            dsl = slice(d * 128, (d + 1) * 128)
            kt, v_sb = kt_sbs[d], v_sbs[d]
            for ch in range(TKV // 512):
                ksl = slice(ch * 512, (ch + 1) * 512)
                k_ps = ppK.tile([128, 512], F32, tag="k", name=f"k{d}{ch}")
                for p in range(CP):
                    nc.tensor.matmul(k_ps, wk8[:, p, :, dsl],
                                     xkv8[:, p, :, ksl], start=(p == 0),
                                     stop=(p == CP - 1), perf_mode=DRM)
                nc.scalar.copy(kt[:, ksl], k_ps)
                v_ps = ppK.tile([128, 4, 128], F32, tag="k", name=f"vp{d}{ch}")
                for j in range(4):
                    ktt = ch * 4 + j
                    ktsl = slice(ktt * 128, (ktt + 1) * 128)
                    for p in range(CP):
                        nc.tensor.matmul(v_ps[:, j, :],
                                         xkv8[:, p, :, ktsl],
                                         wv8[:, p, :, dsl], start=(p == 0),
                                         stop=(p == CP - 1), perf_mode=DRM)
                # [128,4,128] -> v_sb[:, 2ch:2ch+2, :, :, 0:HD]
                nc.vector.tensor_copy(
                    v_sb[:, 2 * ch:2 * ch + 2, :, :, 0:HD],
                    v_ps.rearrange("q (g s) (h x) -> q g s h x", g=2, h=2))

        # ================= attention ======================================
        def attn_one(d, hh, t2):
            qt, kt, v_sb = qt_sbs[d], kt_sbs[d], v_sbs[d]
            tsl = slice(t2 * 512, (t2 + 1) * 512)
            rsl = slice(hh * 64, hh * 64 + 64)
            o_ps = ppo.tile([128, 4, HD + 1], F32, tag="o",
                            name=f"ops{d}{hh}{t2}")
            q_dr = qt[rsl, :, tsl]
            NW = 8                      # groups per wave
            for w in range(KTT // 2 // NW):
                sexps = []
                for gg in range(NW):
                    g = w * NW + gg
                    s_ps = ppS.tile([128, 2, 512], F32, tag="s",
                                    name=f"s{d}{hh}{t2}{g}")
                    for i in range(2):
                        ktt = 2 * g + i
                        kbase = kt[rsl, ktt * 128:(ktt + 1) * 128]
                        k_dr = bass.AP(tensor=kbase.tensor,
                                       offset=kbase.offset,
                                       ap=[kbase.ap[0], [0, 2], kbase.ap[1]])
                        nc.tensor.matmul(s_ps[:, i, :], k_dr, q_dr,
                                         start=True, stop=True, perf_mode=DRM)
                    sexp = sexp_pool.tile([128, 2, 512], FP8, tag="se",
                                          name=f"se{d}{hh}{t2}{g}")
                    sexps.append(sexp)
                    if g % 2 == 0:
                        nc.scalar.activation(sexp, s_ps, AF.Exp, scale=ALPHA)
                    else:
                        nc.vector._custom_dve(EXP_POLY3, out=sexp, in0=s_ps,
                                              s0=P3_C0, s1=P3_C1, imm2=ALPHA)
                for gg in range(NW):
                    g = w * NW + gg
                    for tt in range(4):
                        nc.tensor.matmul(
                            o_ps[:, tt, :],
                            sexps[gg][:, :, tt * 128:(tt + 1) * 128],
                            v_sb[:, g, :, hh, :],
                            start=(g == 0), stop=(g == KTT // 2 - 1),
                            perf_mode=DRM)
            # normalize: o = o_raw * (1/denom), stride-0 reciprocal view
            rr = sm.tile([128, 4], F32, tag="rr", name=f"rr{d}{hh}{t2}")
            nc.vector.reciprocal(rr, o_ps[:, :, HD])
            rrv = bass.AP(tensor=rr.tensor, offset=rr.offset,
                          ap=[rr.ap[0], [1, 4], [0, HD]])
            h = 2 * d + hh
            nc.vector.tensor_tensor(
                o_sb[:, t2 * 4:(t2 + 1) * 4, h * 64:(h + 1) * 64],
                o_ps[:, :, 0:HD], rrv, op=ALU.mult)

        # ================= proj + ReduceScatter ===========================
        rsx_tiles = []

        def proj_rs(t2):
            tsl = slice(t2 * 512, (t2 + 1) * 512)
            # batch transposes: 6 per 1-bank psum tile, one copy each
            for half in range(2):
                t_ps = ppK.tile([128, 6, 128], BF16, tag="k",
                                name=f"tp{t2}{half}")
                for j in range(6):
                    tt = half * 2 + j // 3
                    d = j % 3
                    nc.tensor.transpose(
                        t_ps[:, j, :],
                        o_sb[:, t2 * 4 + tt, d * 128:(d + 1) * 128], ident)
                for j in range(6):
                    tt = half * 2 + j // 3
                    d = j % 3
                    nc.vector.tensor_copy(
                        ot_sb[:, d,
                              t2 * 512 + tt * 128:t2 * 512 + (tt + 1) * 128],
                        t_ps[:, j, :])
            for c in range(CT):
                csl = slice(c * 128, (c + 1) * 128)
                p_ps = ppK.tile([128, 512], F32, tag="k", name=f"pj{c}{t2}")
                for p in range(2):
                    if p == 0:
                        rhs = ot_sb[:, 0:2, tsl]
                    else:
                        rb = ot_sb[:, 2, tsl]
                        rhs = bass.AP(tensor=rb.tensor, offset=rb.offset,
                                      ap=[rb.ap[0], [0, 2], rb.ap[1]])
                    nc.tensor.matmul(p_ps, wp8[:, p, :, csl], rhs,
                                     start=(p == 0), stop=(p == 1),
                                     perf_mode=DRM)
                nc.vector.tensor_copy(x1_sb[:, c, tsl], p_ps)
            for blk in range(2):
                nc.gpsimd.dma_start(
                    cc_in[t2][blk].rearrange("(a p) t -> p a t", p=128),
                    x1_sb[:, :, t2 * 512 + blk * 256:t2 * 512 + (blk + 1) * 256])
            nc.gpsimd.collective_compute(
                "ReduceScatter", ALU.add, ins=[cc_in[t2]], outs=[cc_rs[t2]],
                replica_groups=groups)
            rsx = upool.tile([128, CT, 256], FP8, tag="rsx", name=f"rsx{t2}")
            nc.gpsimd.dma_start(rsx, cc_rs[t2].rearrange("(a p) t -> p a t",
                                                         p=128))
            rsx_tiles.append(rsx)

        # ================= schedule: attention phase ======================
        make_q(0)
        make_kv(0)
        attn_one(0, 0, 0)
        attn_one(0, 0, 1)
        make_q(1)
        make_kv(1)
        attn_one(0, 1, 0)
        attn_one(0, 1, 1)
        make_q(2)
        make_kv(2)
        # xkv8 is dead now; its slot hosts wm28 for the MLP tail
        wm28 = big.tile([128, HP, 2, DIM], FP8, tag="xkv8", name="wm28")
        nc.sync.dma_start(wm28, wm28_d)
        attn_one(1, 0, 0)
        attn_one(1, 1, 0)
        attn_one(2, 0, 0)
        attn_one(2, 1, 0)
        proj_rs(0)

        # ================= post-RS: residual + LN2 + MLP ==================
        h8s = []

        def mlp_q(q):
            qsl = slice(q * 256, (q + 1) * 256)
            rsx = rsx_tiles[q]
            for c in range(CT):
                nc.vector.affine_then_add(x1h[:, c, qsl], rsx[:, c, :],
                                          xqh_sb[:, c, qsl],
                                          scale=INV_SW2,
                                          bias=pb_sb[:, c:c + 1])
            # LN2 stats (bf16 ones-matmuls)
            x2h = upool.tile([128, CT, 256], FP8, tag="x2h", name=f"x2h{q}")
            for c in range(CT):
                nc.gpsimd.tensor_tensor(x2h[:, c, :], x1h[:, c, qsl],
                                        x1h[:, c, qsl], op=ALU.mult)
            s2_ps = ppK.tile([1, 256], F32, tag="k", name=f"s2{q}")
            for c in range(CT):
                nc.tensor.matmul(s2_ps, ones_bf, x1h[:, c, qsl],
                                 start=(c == 0), stop=(c == CT - 1))
            mu2 = sm2.tile([1, 256], F32, tag="st2", name=f"mu2{q}")
            nc.vector.tensor_scalar_mul(mu2, s2_ps, 1.0 / DIM)
            mu2bf = sm2.tile([1, 256], BF16, tag="st2", name=f"mu2bf{q}")
            nc.vector.tensor_copy(mu2bf, mu2)
            mu2_b = bc_pool.tile([128, 256], BF16, tag="bc2", name=f"mu2b{q}")
            nc.gpsimd.partition_broadcast(mu2_b, mu2bf)
            q2_ps = ppK.tile([1, 256], F32, tag="k", name=f"q2{q}")
            for c in range(CT):
                nc.tensor.matmul(q2_ps, ones_bf, x2h[:, c, :],
                                 start=(c == 0), stop=(c == CT - 1))
            m22 = sm2.tile([1, 256], F32, tag="st2", name=f"m22{q}")
            nc.vector.tensor_tensor(m22, mu2, mu2, op=ALU.mult)
            var2 = sm2.tile([1, 256], F32, tag="st2", name=f"var2{q}")
            nc.vector.scalar_tensor_tensor(var2, q2_ps, 1.0 / DIM, m22,
                                           op0=ALU.mult, op1=ALU.subtract)
            lnv2 = sm2.tile([1, 256], F32, tag="st2", name=f"lnv2{q}")
            nc.scalar.activation(lnv2, var2, AF.Ln, bias=eps_t[:1, :],
                                 scale=1.0)
            rs2 = sm2.tile([1, 256], F32, tag="st2", name=f"rs2{q}")
            nc.scalar.activation(rs2, lnv2, AF.Exp, scale=-0.5)
            rs2bf = sm2.tile([1, 256], BF16, tag="st2", name=f"rs2bf{q}")
            nc.vector.tensor_copy(rs2bf, rs2)
            rs2_b = bc_pool.tile([128, 256], BF16, tag="bc2", name=f"rs2b{q}")
            nc.gpsimd.partition_broadcast(rs2_b, rs2bf)

            def bview(t):
                a = t[:, :]
                return bass.AP(tensor=a.tensor, offset=a.offset,
                               ap=[a.ap[0], [0, CT], a.ap[1]])

            tx = upool.tile([128, CT, 256], BF16, tag="u", name=f"tx{q}")
            nc.vector.tensor_tensor(tx, x1h[:, :, qsl], bview(mu2_b),
                                    op=ALU.subtract)
            xn2b = upool.tile([128, CT, 256], BF16, tag="u", name=f"xn2b{q}")
            nc.vector.tensor_tensor(xn2b, tx, bview(rs2_b), op=ALU.mult)
            xn2 = upool.tile([128, CP, 2, 256], FP8, tag="xn2", name=f"xn2{q}")
            nc.gpsimd.tensor_copy(xn2.rearrange("r p s t -> r (p s) t"), xn2b)
            xn2r = upool.tile([128, CP, 2, 256], FP8, tag="xn2r",
                              name=f"xn2r{q}")
            nc.gpsimd.tensor_tensor(xn2r.rearrange("r p s t -> r (p s) t"),
                                    xn2b,
                                    xn2.rearrange("r p s t -> r (p s) t"),
                                    op=ALU.subtract)

            # reuse dead slots: q0 -> xq8 (dead after make_q/LN1),
            # q1 -> o_sb (dead after proj(1) transposes)
            if q == 0:
                h8 = big.tile([128, HP, 2, 256], FP8, tag="xq8", name="h8q0")
            else:
                h8 = med.tile([128, HP, 2, 256], FP8, tag="osb", name="h8q1")
            h8s.append(h8)
            for mg in range(HP // 2):
                h_ps = ppS.tile([128, 4, 256], F32, tag="s", name=f"h{q}{mg}")
                for mi in range(4):
                    m = 4 * mg + mi
                    msl = slice(m * 128, (m + 1) * 128)
                    nc.tensor.matmul(h_ps[:, mi, :], b1p8[:, m, :, :], ones8r,
                                     start=True, stop=False, perf_mode=DRM)
                    for p in range(CP):
                        nc.tensor.matmul(h_ps[:, mi, :], wm18[:, p, :, msl],
                                         xn2[:, p, :, :], start=False,
                                         stop=False, perf_mode=DRM)
                    for p in range(CP):
                        nc.tensor.matmul(h_ps[:, mi, :], wm18r[:, p, :, msl],
                                         xn2[:, p, :, :], start=False,
                                         stop=False, perf_mode=DRM)
                    for p in range(CP):
                        nc.tensor.matmul(h_ps[:, mi, :], wm18[:, p, :, msl],
                                         xn2r[:, p, :, :], start=False,
                                         stop=(p == CP - 1), perf_mode=DRM)
                nc.scalar.activation(
                    h8[:, 2 * mg:2 * mg + 2, :, :],
                    h_ps.rearrange("r (a s) t -> r a s t", a=2), AF.Gelu,
                    scale=INV_SW)

        def mlp2_q(q):
            qsl = slice(q * 256, (q + 1) * 256)
            h8 = h8s[q]
            for cp in range(CP):
                o2_ps = ppK.tile([128, 2, 256], F32, tag="k",
                                 name=f"o2{q}{cp}")
                for ci in range(2):
                    c = 2 * cp + ci
                    csl = slice(c * 128, (c + 1) * 128)
                    wm2rc = upool.tile([128, HP, 2, 128], FP8, tag="wm2rc",
                                       name=f"wm2rc{q}{cp}{ci}")
                    nc.sync.dma_start(wm2rc, wm28r_d[:, :, :, csl])
                    for hp in range(HP):
                        nc.tensor.matmul(o2_ps[:, ci, :],
                                         wm28[:, hp, :, csl],
                                         h8[:, hp, :, :], start=(hp == 0),
                                         stop=False, perf_mode=DRM)
                    for hp in range(HP):
                        nc.tensor.matmul(o2_ps[:, ci, :],
                                         wm2rc[:, hp, :, :],
                                         h8[:, hp, :, :], start=False,
                                         stop=(hp == HP - 1), perf_mode=DRM)
                for ci in range(2):
                    c = 2 * cp + ci
                    csl = slice(c * 128, (c + 1) * 128)
                    fin = outp.tile([128, 256], F32, tag="outp",
                                    name=f"fin{q}{cp}{ci}")
                    nc.vector.affine_then_add(fin, o2_ps[:, ci, :],
                                              x1h[:, c, qsl], scale=INV_SW,
                                              bias=b2_sb[:, c:c + 1])
                    nc.sync.dma_start(out_d[csl, qsl], fin)

        attn_one(1, 0, 1)
        attn_one(1, 1, 1)
        attn_one(2, 0, 1)
        attn_one(2, 1, 1)
        proj_rs(1)
        mlp_q(0)
        mlp2_q(0)
        mlp_q(1)
        mlp2_q(1)

    nc.compile()
    return nc


def _prep_inputs(inputs):
    """Build the 8 per-core in_maps from the full-size inputs."""
    f8 = np.float64
    xq = np.asarray(inputs["xq"], np.float32)
    xkv = np.asarray(inputs["xkv"], np.float32)
    n1w = np.asarray(inputs["norm1_w"], f8); n1b = np.asarray(inputs["norm1_b"], f8)
    kv_w = np.asarray(inputs["kv_w"], f8); kv_b = np.asarray(inputs["kv_b"], f8)
    q_w = np.asarray(inputs["q_w"], f8); q_b = np.asarray(inputs["q_b"], f8)
    p_w = np.asarray(inputs["proj_w"], f8); p_b = np.asarray(inputs["proj_b"], f8)
    n2w = np.asarray(inputs["norm2_w"], f8); n2b = np.asarray(inputs["norm2_b"], f8)
    w1 = np.asarray(inputs["mlp_w1"], f8); b1 = np.asarray(inputs["mlp_b1"], f8)
    w2 = np.asarray(inputs["mlp_w2"], f8); b2 = np.asarray(inputs["mlp_b2"], f8)

    def cpair(mat, ncols):
        """[768, ncols] -> [128, 3, 2, ncols] channel-pair layout."""
        return np.ascontiguousarray(
            mat.reshape(CP, 2, 128, ncols).transpose(2, 0, 1, 3))

    wq_f = n1w[:, None] * q_w                                # [768, 768]
    qb_f = (q_b + n1b @ q_w) * SW                            # [768]
    kvw = kv_w.reshape(DIM, 2, NH, HD)
    kvb = kv_b.reshape(2, NH, HD)
    # v-bias folded into proj bias: sum over ALL heads
    pb_f = p_b + kvb[1].reshape(NH * HD) @ p_w               # [768]
    wm1_f = n2w[:, None] * w1
    b1_f = b1 + n2b @ w1
    wm18 = cpair((SW * wm1_f), MLPD).astype(F8)
    wm18r = cpair(
        SW * wm1_f - cpair(SW * wm1_f, MLPD).astype(F8).astype(np.float64)
        .transpose(1, 2, 0, 3).reshape(DIM, MLPD), MLPD).astype(F8)
    wm28 = np.ascontiguousarray(
        (SW * w2).reshape(HP, 2, 128, DIM).transpose(2, 0, 1, 3)).astype(F8)
    wm28r = np.ascontiguousarray(
        ((SW * w2) - (SW * w2).astype(F8).astype(np.float64))
        .reshape(HP, 2, 128, DIM).transpose(2, 0, 1, 3)).astype(F8)
    b1p8 = np.zeros((1, HT, 2, 128), F8)
    b1p8[0, :, 0, :] = (SW * b1_f).reshape(HT, 128).astype(F8)

    maps = []
    for core in range(NCORES):
        b, half = divmod(core, 2)
        hs = slice(half * NHL, (half + 1) * NHL)
        tidx = np.r_[half * 256:(half + 1) * 256,
                     512 + half * 256:512 + (half + 1) * 256]
        xqT = np.ascontiguousarray(xq[b].reshape(TQ, DIM).T)
        xkvT = np.ascontiguousarray(xkv[b].reshape(TKV, DIM).T)
        wq_c = cpair(SW * np.ascontiguousarray(
            wq_f.reshape(DIM, NH, HD)[:, hs].reshape(DIM, DL)), DL).astype(F8)
        wp_loc = SW * p_w.reshape(NH, HD, DIM)[hs].reshape(DL, DIM)
        wp_pad = np.zeros((4, 128, DIM), f8)
        wp_pad[0:DT] = wp_loc.reshape(DT, 128, DIM)
        wp8 = np.ascontiguousarray(
            wp_pad.reshape(2, 2, 128, DIM).transpose(2, 0, 1, 3)).astype(F8)
        m = {
            "xq8": cpair(xqT, TQ).astype(F8),
            "xqh": np.ascontiguousarray(
                xqT[:, tidx].reshape(CT, 128, TH).transpose(1, 0, 2)
            ).astype(BF),
            "xkv8": cpair(xkvT, TKV).astype(F8),
            "wq8": wq_c,
            "wk8": cpair(SW * np.ascontiguousarray(
                kvw[:, 0, hs].reshape(DIM, DL)), DL).astype(F8),
            "wv8": cpair(SW * np.ascontiguousarray(
                kvw[:, 1, hs].reshape(DIM, DL)), DL).astype(F8),
            "wp8": wp8,
            "wm18": wm18,
            "wm18r": wm18r,
            "wm28": wm28,
            "wm28r": wm28r,
            "b1p8": b1p8,
            # sq: column sums of the fp8 wq (exact, f64), [DL]
            "sq": wq_c.astype(np.float64).sum(axis=(0, 1, 2)).astype(
                np.float32),
            "qb": np.ascontiguousarray(
                qb_f.reshape(NH, HD)[hs].reshape(DL)).astype(np.float32),
            "pb": pb_f.astype(np.float32),
            "b2": b2.astype(np.float32),
        }
        maps.append(m)
    return maps


def kernel(**inputs):
    if "nc" not in _CACHE:
        _CACHE["nc"] = _build_program()
    nc = _CACHE["nc"]
    maps = _prep_inputs(inputs)
    res = run_bass_kernel_spmd(nc, maps, core_ids=list(range(NCORES)))
    out = np.zeros((B, TQ, DIM), np.float32)
    for core in range(NCORES):
        b, half = divmod(core, 2)
        tidx = np.r_[half * 256:(half + 1) * 256,
                     512 + half * 256:512 + (half + 1) * 256]
        x2T = res.results[core]["out"]          # [768, 512]
        out[b, tidx, :] = x2T.T
    return out.reshape(B, 32, 32, DIM)
